# revision 1
# baseline (speedup 1.0000x reference)
"""DiversityAttention on 8 TRN2 NeuronCores (Bass/Tile).

Sharding: data-parallel over batch (B=2) x tensor-parallel over heads
(16 heads -> 4 groups of 4). core = (b, g), b = core // 4, g = core % 4.
Each core computes full attention for its 4 heads over its batch and a
partial out-projection [S, HIDDEN]; the host sums the 4 partials per
batch and adds bo.

Device-side formulation (keys-on-partitions / "S^T" orientation, so no
large transposes are ever needed):
  qT = (Wq/8 @ x^T + bq/8)   [64h, S]   (1/sqrt(dh) folded into Wq on host)
  kT = (Wk   @ x^T + bk)     [64h, S]
  vT = (Wv   @ x^T + bv) then PE-transposed to V [S, 64h] (+ ones col)
  xh = sqrt(gamma) * x^T / max(||x||, eps)  (in-place columns scale of xT)
  per (ktile, qblock): sim_psum[k,q] = xh^T xh ; per head:
     scores_psum[k,q] = kT^T qT ; P = exp(scores - sim) (DVE sub, ACT exp)
  ctx^T[d,q] (+sums row) = sum_k [V|1]^T P  accumulated in PSUM
  ctx normalized by reciprocal(sums) (PE broadcast outer product)
  out[q,o] partial = ctxT^T @ WoT  -> DMA to DRAM

All matmuls run as float32r (full PE rate at N>=256, ~fp32 precision).
"""

import math
import os
import sys

import numpy as np

for _p in ("/opt/trn_rl_repo",):
    if _p not in sys.path and os.path.isdir(_p):
        sys.path.insert(0, _p)

os.environ.setdefault("MYCRO_LOCAL_CACHE", "1")

import concourse.bass as bass
import concourse.tile as tile
from concourse import bacc, mybir
from concourse.bass_utils import run_bass_kernel_spmd
from concourse.masks import make_identity


def _install_ntff_hook():
    """Provide antenv.axon_hooks (NTFF profiling registry) if the image
    lacks it, mirroring trn_agent_boot's ctypes hook. No-op on failure."""
    try:
        import antenv.axon_hooks  # noqa: F401
        return
    except ImportError:
        pass
    try:
        import contextlib
        import ctypes
        import types

        so_path = "/opt/axon/libaxon_pjrt.so"
        if not os.path.exists(so_path):
            return
        lib = ctypes.CDLL(so_path)
        if not hasattr(lib, "axon_start_nrt_profile"):
            return
        lib.axon_start_nrt_profile.argtypes = [
            ctypes.POINTER(ctypes.c_int64), ctypes.c_size_t]
        lib.axon_start_nrt_profile.restype = ctypes.c_int64
        lib.axon_stop_nrt_profile.argtypes = [ctypes.c_char_p]
        lib.axon_stop_nrt_profile.restype = ctypes.c_int64

        @contextlib.contextmanager
        def _hook(output_dir, device_ids):
            import jax
            jax.devices()
            if device_ids:
                ids = (ctypes.c_int64 * len(device_ids))(*device_ids)
                rc = lib.axon_start_nrt_profile(ids, len(device_ids))
            else:
                rc = lib.axon_start_nrt_profile(None, 0)
            if rc != 0:
                raise RuntimeError(f"axon_start_nrt_profile rc={rc}")
            try:
                yield
            finally:
                n = lib.axon_stop_nrt_profile(str(output_dir).encode())
                print(f"ntff profile: {n} file(s) -> {output_dir}",
                      file=sys.stderr)

        mod = types.ModuleType("antenv.axon_hooks")
        _state = {"hook": _hook}
        mod.set_axon_ntff_profile_hook = lambda h: _state.__setitem__("hook", h)
        mod.get_axon_ntff_profile_hook = lambda: _state["hook"]
        sys.modules["antenv.axon_hooks"] = mod
        import antenv
        antenv.axon_hooks = mod
    except Exception:
        pass


_install_ntff_hook()

F32 = mybir.dt.float32
F32R = mybir.dt.float32r
ALU = mybir.AluOpType
ACT_EXP = mybir.ActivationFunctionType.Exp
ACT_COPY = mybir.ActivationFunctionType.Copy

# Problem constants (hardcoded per contract).
HIDDEN = 1024
HEADS = 16
HEAD_DIM = 64
GAMMA = 0.5
B, S = 2, 2048
N_CORES = 8
GROUPS = N_CORES // B  # head groups per batch
HPC = HEADS // GROUPS  # heads per core
LAG = 2  # kt software-pipeline lag between exp and ctx matmul


def _r(ap):
    return ap.bitcast(F32R)


def emit_kernel(tc, aps, *, S_, C_, HPC_, QB):
    """Emit the per-core kernel. aps: dict of dram APs."""
    nc = tc.nc
    CT = C_ // 128          # contraction tiles over hidden
    PAIRS = HPC_ // 2       # head pairs (128-channel chunks)
    NKT = S_ // 128         # key tiles
    NQB = S_ // QB          # query blocks
    PB = min(512, S_)       # projection free-block width
    NPB = S_ // PB
    OB_W = min(512, C_)     # out-projection free-block width
    NOB = C_ // OB_W

    xT_d = aps["xT"]; scale_d = aps["scale"]
    wq_d = aps["wq"]; wk_d = aps["wk"]; wv_d = aps["wv"]; wo_d = aps["wo"]
    bq_d = aps["bq"]; bk_d = aps["bk"]; bv_d = aps["bv"]
    out_d = aps["out"]
    mask_d = aps.get("maskadd")

    from contextlib import ExitStack
    stack = ExitStack()
    consts = stack.enter_context(tc.tile_pool(name="consts", bufs=1))
    xpool = stack.enter_context(tc.tile_pool(name="xpool", bufs=1))
    projpool = stack.enter_context(tc.tile_pool(name="projpool", bufs=1))

    # --- constants ---
    identity = consts.tile([128, 128], F32)
    make_identity(nc, identity)

    wo_sb = consts.tile([128, PAIRS, C_], F32R)

    # x^T loaded in chunks and rounded to fp32r by the scalar engine
    xTr = xpool.tile([128, CT, S_], F32R)

    # projections (fp32r for q/k; plain f32 for v which feeds the transpose)
    qT_sb = projpool.tile([128, PAIRS, S_], F32R)
    kT_sb = projpool.tile([128, PAIRS, S_], F32R)
    v2_sb = projpool.tile([128, HPC_, NKT, HEAD_DIM + 1], F32R)

    with tc.tile_pool(name="xstage", bufs=2) as xstage, \
         tc.tile_pool(name="wstage", bufs=1) as wstage, \
         tc.tile_pool(name="wpool", bufs=1) as wpool, \
         tc.tile_pool(name="vstage", bufs=1) as vstage, \
         tc.tile_pool(name="ph1psum", bufs=2, space="PSUM") as prj_ps, \
         tc.tile_pool(name="tppsum", bufs=4, space="PSUM") as tp_ps:
        # load + round x^T
        for c in range(CT):
            xs = xstage.tile([128, S_], F32, tag="xs")
            nc.sync.dma_start(out=xs, in_=xT_d[c * 128:(c + 1) * 128, :])
            nc.scalar.activation(out=xTr[:, c, :], in_=xs, func=ACT_COPY)
        # load + round weights (DVE)
        wq_sb = wpool.tile([128, CT, D2_of(HPC_)], F32R)
        wk_sb = wpool.tile([128, CT, D2_of(HPC_)], F32R)
        wv_sb = wpool.tile([128, CT, D2_of(HPC_)], F32R)
        for w_sb, w_d in ((wq_sb, wq_d), (wk_sb, wk_d), (wv_sb, wv_d)):
            ws = wstage.tile([128, CT, D2_of(HPC_)], F32, tag="ws")
            nc.sync.dma_start(out=ws, in_=w_d.rearrange("(t p) m -> p t m", p=128))
            nc.vector.tensor_copy(w_sb, ws)
        wos = wstage.tile([128, PAIRS, C_], F32, tag="ws")
        nc.sync.dma_start(out=wos, in_=wo_d.rearrange("(j p) o -> p j o", p=128))
        nc.vector.tensor_copy(wo_sb, wos)
        bq_sb = wpool.tile([128, PAIRS, 1], F32)
        bk_sb = wpool.tile([128, PAIRS, 1], F32)
        bv_sb = wpool.tile([128, PAIRS, 1], F32)
        for b_sb, b_d in ((bq_sb, bq_d), (bk_sb, bk_d), (bv_sb, bv_d)):
            nc.sync.dma_start(
                out=b_sb, in_=b_d.rearrange("(j p) one -> p j one", p=128))

        vT_sb = vstage.tile([128, PAIRS, S_], F32)
        for w_sb, b_sb, dest in (
            (wq_sb, bq_sb, qT_sb),
            (wk_sb, bk_sb, kT_sb),
            (wv_sb, bv_sb, vT_sb),
        ):
            for nb in range(NPB):
                pss = [prj_ps.tile([128, PB], F32, tag=f"prj{j}",
                                   name=f"prj_{dest.tensor.name}_{nb}_{j}")
                       for j in range(PAIRS)]
                for c in range(CT):
                    for j in range(PAIRS):
                        nc.tensor.matmul(
                            pss[j],
                            w_sb[:, c, j * 128:(j + 1) * 128],
                            xTr[:, c, nb * PB:(nb + 1) * PB],
                            start=(c == 0),
                            stop=(c == CT - 1),
                        )
                for j in range(PAIRS):
                    nc.vector.tensor_scalar_add(
                        dest[:, j, nb * PB:(nb + 1) * PB], pss[j], b_sb[:, j, :]
                    )

        # V: PE-transpose vT (f32) -> [keys, d] layout, 2 heads per tile
        for j in range(PAIRS):
            for t in range(NKT):
                tp = tp_ps.tile([128, 128], F32, tag="tp")
                nc.tensor.transpose(tp, vT_sb[:, j, t * 128:(t + 1) * 128], identity)
                nc.scalar.activation(
                    out=v2_sb[:, 2 * j:2 * j + 2, t, 0:HEAD_DIM],
                    in_=tp.rearrange("p (h d) -> p h d", h=2),
                    func=ACT_COPY,
                )
        onescol = wstage.tile([128, HPC_, NKT, 1], F32)
        nc.vector.memset(onescol, 1.0)
        nc.vector.tensor_copy(v2_sb[:, :, :, HEAD_DIM:HEAD_DIM + 1], onescol)

    # xT -> xh in place: multiply columns by sqrt(gamma)/||x_row||
    ctxT2_sb = projpool.tile([128, PAIRS, S_], F32R)
    with tc.tile_pool(name="bcpool", bufs=1) as bcpool:
        bcast_sb = bcpool.tile([128, S_], F32)
        nc.sync.dma_start(out=bcast_sb, in_=scale_d.to_broadcast([128, S_]))
        for c in range(CT):
            nc.vector.tensor_mul(xTr[:, c, :], xTr[:, c, :], bcast_sb)

    # --- main loop (phase 2) ---
    ptpool = stack.enter_context(tc.tile_pool(name="ptpool", bufs=7))
    spool = stack.enter_context(tc.tile_pool(name="spool", bufs=2))
    simsb = stack.enter_context(tc.tile_pool(name="simsb", bufs=2))
    smallpool = stack.enter_context(tc.tile_pool(name="smallpool", bufs=2))
    mpool = (stack.enter_context(tc.tile_pool(name="mpool", bufs=2))
             if mask_d is not None else None)

    with tc.tile_pool(name="simpsum", bufs=2, space="PSUM") as simp, \
         tc.tile_pool(name="scpsum", bufs=1, space="PSUM") as scp, \
         tc.tile_pool(name="ctxpsum", bufs=1, space="PSUM") as ctxp:

        def emit_ctx(ctx_ps, kt, pt_pairs):
            for j in range(PAIRS):
                for hi in range(2):
                    nc.tensor.matmul(
                        ctx_ps[2 * j + hi],
                        v2_sb[:, 2 * j + hi, kt, :],
                        pt_pairs[j][:, hi, :],
                        start=(kt == 0),
                        stop=(kt == NKT - 1),
                        skip_group_check=True,
                    )

        def emit_division_head(qb, ctx_ps, h):
            j, hi = divmod(h, 2)
            r0 = smallpool.tile([1, QB], F32, tag=f"r0{h % 2}",
                                name=f"r0_{qb}_{h}")
            nc.vector.reciprocal(
                r0, ctx_ps[h][HEAD_DIM:HEAD_DIM + 1, :])
            rb = smallpool.tile([HEAD_DIM, QB], F32, tag="rb")
            nc.gpsimd.partition_broadcast(rb, r0, channels=HEAD_DIM)
            nc.vector.tensor_mul(
                ctxT2_sb[hi * 64:hi * 64 + 64, j, qb * QB:(qb + 1) * QB],
                ctx_ps[h][0:HEAD_DIM, :],
                rb,
            )

        def emit_division(qb, ctx_ps):
            for h in range(HPC_):
                emit_division_head(qb, ctx_ps, h)

        prev_div = None
        for qb in range(NQB):
            ctx_ps = [ctxp.tile([HEAD_DIM + 1, QB], F32, tag=f"ctx{h}",
                                name=f"ctx_{qb}_{h}")
                      for h in range(HPC_)]
            pending = []
            for kt in range(NKT):
                if prev_div is not None and kt >= 2 and (kt - 2) % 3 == 0:
                    h = (kt - 2) // 3
                    if h < HPC_:
                        emit_division_head(prev_div[0], prev_div[1], h)
                        if h == HPC_ - 1:
                            prev_div = None
                sp = simp.tile([128, QB], F32, tag="sim")
                for c in range(CT):
                    nc.tensor.matmul(
                        sp,
                        xTr[:, c, kt * 128:(kt + 1) * 128],
                        xTr[:, c, qb * QB:(qb + 1) * QB],
                        start=(c == 0),
                        stop=(c == CT - 1),
                    )
                sim_t = simsb.tile([128, QB], F32, tag="simsb")
                nc.scalar.activation(out=sim_t, in_=sp, func=ACT_COPY)
                if mask_d is not None:
                    m_sb = mpool.tile([128, QB], F32, tag="msk")
                    nc.sync.dma_start(
                        out=m_sb,
                        in_=mask_d[kt * 128:(kt + 1) * 128, qb * QB:(qb + 1) * QB],
                    )
                    nc.vector.tensor_sub(sim_t, sim_t, m_sb)
                pt_pairs = []
                for j in range(PAIRS):
                    sc_t = scp.tile([128, 2, QB], F32, tag="scp")
                    for hi in range(2):
                        pr = slice(hi * 64, hi * 64 + 64)
                        nc.tensor.matmul(
                            sc_t[:, hi, :],
                            kT_sb[pr, j, kt * 128:(kt + 1) * 128],
                            qT_sb[pr, j, qb * QB:(qb + 1) * QB],
                            start=True,
                            stop=True,
                        )
                    s_t = spool.tile([128, 2, QB], F32, tag="s")
                    nc.vector.tensor_sub(
                        s_t, sc_t,
                        sim_t.unsqueeze(1).to_broadcast([128, 2, QB]))
                    pt = ptpool.tile([128, 2, QB], F32R, tag="pt")
                    nc.scalar.activation(out=pt, in_=s_t, func=ACT_EXP)
                    pt_pairs.append(pt)
                pending.append((kt, pt_pairs))
                if len(pending) > LAG:
                    k0, p0 = pending.pop(0)
                    emit_ctx(ctx_ps, k0, p0)
            for k0, p0 in pending:
                emit_ctx(ctx_ps, k0, p0)
            if prev_div is not None:
                done = max(0, (NKT - 1 - 2) // 3 + 1) if NKT > 2 else 0
                for h in range(min(done, HPC_), HPC_):
                    emit_division_head(prev_div[0], prev_div[1], h)
                prev_div = None
            prev_div = (qb, ctx_ps)
        emit_division(*prev_div)

    # --- out-projection (phase 3) ---
    with tc.tile_pool(name="outpsum", bufs=4, space="PSUM") as outp, \
         tc.tile_pool(name="outstg", bufs=4) as outstg:
        for qt in range(S_ // 128):
            for ob in range(NOB):
                op = outp.tile([128, OB_W], F32, tag="op")
                for j in range(PAIRS):
                    nc.tensor.matmul(
                        op,
                        ctxT2_sb[:, j, qt * 128:(qt + 1) * 128],
                        wo_sb[:, j, ob * OB_W:(ob + 1) * OB_W],
                        start=(j == 0),
                        stop=(j == PAIRS - 1),
                    )
                ostg = outstg.tile([128, OB_W], F32, tag="ostg")
                if (qt + ob) % 2 == 0:
                    nc.scalar.activation(out=ostg, in_=op, func=ACT_COPY)
                else:
                    nc.vector.tensor_copy(ostg, op)
                nc.sync.dma_start(
                    out=out_d[qt * 128:(qt + 1) * 128, ob * OB_W:(ob + 1) * OB_W],
                    in_=ostg,
                )

    stack.close()


def D2_of(hpc):
    return hpc * HEAD_DIM


def build_nc(*, S_=S, C_=HIDDEN, HPC_=HPC, QB=512, with_mask=False,
             enable_asserts=False):
    nc = bacc.Bacc(
        "TRN2", target_bir_lowering=False, debug=False,
        enable_asserts=enable_asserts,
    )
    D2 = HPC_ * HEAD_DIM
    aps = {}
    aps["xT"] = nc.dram_tensor("xT", [C_, S_], F32, kind="ExternalInput").ap()
    aps["scale"] = nc.dram_tensor("scale", [1, S_], F32, kind="ExternalInput").ap()
    for n in ("wq", "wk", "wv"):
        aps[n] = nc.dram_tensor(n, [C_, D2], F32, kind="ExternalInput").ap()
    aps["wo"] = nc.dram_tensor("wo", [D2, C_], F32, kind="ExternalInput").ap()
    for n in ("bq", "bk", "bv"):
        aps[n] = nc.dram_tensor(n, [D2, 1], F32, kind="ExternalInput").ap()
    if with_mask:
        aps["maskadd"] = nc.dram_tensor(
            "maskadd", [S_, S_], F32, kind="ExternalInput").ap()
    aps["out"] = nc.dram_tensor("out", [S_, C_], F32, kind="ExternalOutput").ap()

    with tile.TileContext(nc) as tc:
        emit_kernel(tc, aps, S_=S_, C_=C_, HPC_=HPC_, QB=QB)
    nc.compile()
    return nc


def host_prepare(x, attn_mask, Wq, bq, Wk, bk, Wv, bv, Wo, bo, *,
                 S_=S, C_=HIDDEN, HPC_=HPC, n_cores=N_CORES):
    """Build the per-core input maps. Returns (in_maps, with_mask)."""
    x = np.asarray(x, np.float32)
    B_ = x.shape[0]
    groups = n_cores // B_
    Wq = np.asarray(Wq, np.float32); Wk = np.asarray(Wk, np.float32)
    Wv = np.asarray(Wv, np.float32); Wo = np.asarray(Wo, np.float32)
    bq = np.asarray(bq, np.float32); bk = np.asarray(bk, np.float32)
    bv = np.asarray(bv, np.float32)

    inv_sqrt_d = 1.0 / math.sqrt(HEAD_DIM)
    WqT = np.ascontiguousarray((Wq * inv_sqrt_d).T)  # [C, C] in->out
    WkT = np.ascontiguousarray(Wk.T)
    WvT = np.ascontiguousarray(Wv.T)
    WoT = np.ascontiguousarray(Wo.T)                 # [C(c), C(o)]
    bq = bq * inv_sqrt_d

    mask = np.asarray(attn_mask)
    with_mask = bool(mask.any())
    maskadd = None
    if with_mask:
        # reference: where(mask, -inf); use a large negative additive bias
        maskadd = np.where(mask, np.float32(-1e30), np.float32(0.0)).astype(np.float32)
        # device layout: maskadd[k, q] added to scores^T
        maskadd = np.ascontiguousarray(maskadd.T)  # [k, q] = mask[q, k].T

    in_maps = []
    for core in range(n_cores):
        b, g = divmod(core, groups)
        xb = x[b]                                   # [S, C]
        xT = np.ascontiguousarray(xb.T)             # [C, S]
        norms = np.linalg.norm(xb, axis=1)          # [S]
        scale = (math.sqrt(GAMMA) / np.maximum(norms, 1e-12)).astype(np.float32)
        ch = slice(g * HPC_ * HEAD_DIM, (g + 1) * HPC_ * HEAD_DIM)
        m = {
            "xT": xT,
            "scale": scale.reshape(1, S_),
            "wq": np.ascontiguousarray(WqT[:, ch]),
            "wk": np.ascontiguousarray(WkT[:, ch]),
            "wv": np.ascontiguousarray(WvT[:, ch]),
            "wo": np.ascontiguousarray(WoT[ch, :]),
            "bq": np.ascontiguousarray(bq[ch]).reshape(-1, 1),
            "bk": np.ascontiguousarray(bk[ch]).reshape(-1, 1),
            "bv": np.ascontiguousarray(bv[ch]).reshape(-1, 1),
        }
        if with_mask:
            m["maskadd"] = maskadd
        in_maps.append(m)
    return in_maps, with_mask


_NC_CACHE = {}


def _get_nc(with_mask):
    key = with_mask
    if key not in _NC_CACHE:
        _NC_CACHE[key] = build_nc(with_mask=with_mask)
    return _NC_CACHE[key]


LAST_RESULTS = None


def kernel(**inputs):
    global LAST_RESULTS
    in_maps, with_mask = host_prepare(
        inputs["x"], inputs["attn_mask"],
        inputs["Wq"], inputs["bq"], inputs["Wk"], inputs["bk"],
        inputs["Wv"], inputs["bv"], inputs["Wo"], inputs["bo"],
    )
    nc = _get_nc(with_mask)
    res = run_bass_kernel_spmd(nc, in_maps, core_ids=list(range(N_CORES)))
    LAST_RESULTS = res
    bo = np.asarray(inputs["bo"], np.float32)
    out = np.zeros((B, S, HIDDEN), np.float32)
    groups = N_CORES // B
    for core in range(N_CORES):
        b = core // groups
        out[b] += res.results[core]["out"]
    out += bo[None, None, :]
    return out



# revision 15
# speedup vs baseline: 1.3307x; 1.3307x over previous
"""DiversityAttention on 8 TRN2 NeuronCores (Bass/Tile), bf16/fp8 edition.

Sharding: data-parallel over batch (B=2) x tensor-parallel over heads
(16 heads -> 4 groups of 4). core = (b, g), b = core // 4, g = core % 4.
Each core computes full attention for its 4 heads over its batch and a
partial out-projection [S, HIDDEN]; the host sums the 4 partials per
batch and adds bo.

Device-side formulation (keys-on-partitions / "S^T" orientation):
  qT = (Wq/sqrt(dh) @ x^T + bq')  [64h, S]  bf16
  kT = (Wk @ x^T + bk)            [64h, S]  bf16
  vT = (Wv @ x^T + bv) then PE-transposed to V [S, 64h] bf16 (+ ones col)
  xh8 = fp8(64 * x^T / max(||x||, eps))  (host-precomputed)
  per (qb, kt):
     sim_psum[k,q] = xh8^T xh8   (fp8 DoubleRow matmuls, 2 chunks/pass)
     En = exp(-gamma/4096 * sim_psum)            (ACT, bf16)
     per head: sc_psum[k,q] = kT^T qT            (bf16 matmul)
               Es = exp(sc_psum)                 (ACT, bf16)
               P  = Es * En                      (DVE 2x bf16)
     ctx^T[d,q] (+sums row) = sum_k [V|1]^T P    (bf16 matmul, PSUM accum)
  ctx normalized by reciprocal_approx_fast(sums), broadcast via gpsimd
  out[q,o] partial = ctxT^T @ WoT (bf16)  -> DMA to DRAM (f32)
"""

import math
import os
import sys

import numpy as np

for _p in ("/opt/trn_rl_repo",):
    if _p not in sys.path and os.path.isdir(_p):
        sys.path.insert(0, _p)

os.environ.setdefault("MYCRO_LOCAL_CACHE", "1")

import ml_dtypes

import concourse.bass as bass
import concourse.tile as tile
from concourse import bacc, mybir
from concourse.bass_utils import run_bass_kernel_spmd
from concourse.masks import make_identity


def _install_ntff_hook():
    """Provide antenv.axon_hooks (NTFF profiling registry) if the image
    lacks it, mirroring trn_agent_boot's ctypes hook. No-op on failure."""
    try:
        import antenv.axon_hooks  # noqa: F401
        return
    except ImportError:
        pass
    try:
        import contextlib
        import ctypes
        import types

        so_path = "/opt/axon/libaxon_pjrt.so"
        if not os.path.exists(so_path):
            return
        lib = ctypes.CDLL(so_path)
        if not hasattr(lib, "axon_start_nrt_profile"):
            return
        lib.axon_start_nrt_profile.argtypes = [
            ctypes.POINTER(ctypes.c_int64), ctypes.c_size_t]
        lib.axon_start_nrt_profile.restype = ctypes.c_int64
        lib.axon_stop_nrt_profile.argtypes = [ctypes.c_char_p]
        lib.axon_stop_nrt_profile.restype = ctypes.c_int64

        @contextlib.contextmanager
        def _hook(output_dir, device_ids):
            import jax
            jax.devices()
            if device_ids:
                ids = (ctypes.c_int64 * len(device_ids))(*device_ids)
                rc = lib.axon_start_nrt_profile(ids, len(device_ids))
            else:
                rc = lib.axon_start_nrt_profile(None, 0)
            if rc != 0:
                raise RuntimeError(f"axon_start_nrt_profile rc={rc}")
            try:
                yield
            finally:
                n = lib.axon_stop_nrt_profile(str(output_dir).encode())
                print(f"ntff profile: {n} file(s) -> {output_dir}",
                      file=sys.stderr)

        mod = types.ModuleType("antenv.axon_hooks")
        _state = {"hook": _hook}
        mod.set_axon_ntff_profile_hook = lambda h: _state.__setitem__("hook", h)
        mod.get_axon_ntff_profile_hook = lambda: _state["hook"]
        sys.modules["antenv.axon_hooks"] = mod
        import antenv
        antenv.axon_hooks = mod
    except Exception:
        pass


_install_ntff_hook()

F32 = mybir.dt.float32
BF16 = mybir.dt.bfloat16
FP8 = mybir.dt.float8e4
ALU = mybir.AluOpType
ACT_EXP = mybir.ActivationFunctionType.Exp
ACT_COPY = mybir.ActivationFunctionType.Copy
DR = mybir.MatmulPerfMode.DoubleRow

# Problem constants (hardcoded per contract).
HIDDEN = 1024
HEADS = 16
HEAD_DIM = 64
GAMMA = 0.5
B, S = 2, 2048
N_CORES = 8
GROUPS = N_CORES // B  # head groups per batch
HPC = HEADS // GROUPS  # heads per core
LAG = 2  # kt software-pipeline lag between P and ctx matmul
FP8_SCALE = 64.0  # host scales normalized x by this before fp8 cast
VW = 66  # v2 padded row width (64 dims + ones col + pad)


def emit_kernel(tc, aps, *, S_, C_, HPC_, QB, with_mask, debug_taps=False):
    """Emit the per-core kernel. aps: dict of dram APs."""
    nc = tc.nc
    CT = C_ // 128          # contraction chunks over hidden
    PAIRS = HPC_ // 2       # head pairs (128-channel chunks)
    NKT = S_ // 128         # key tiles
    NQB = S_ // QB          # query blocks
    PB = min(512, S_)       # projection free-block width
    NPB = S_ // PB
    OB_W = min(512, C_)     # out-projection free-block width
    NOB = C_ // OB_W
    D2 = HPC_ * HEAD_DIM
    neg_gamma_scale = -GAMMA / (FP8_SCALE * FP8_SCALE)

    xbf_d = aps["xbf"]; xh8_d = aps["xh8"]
    wq_d = aps["wq"]; wk_d = aps["wk"]; wv_d = aps["wv"]; wo_d = aps["wo"]
    bq_d = aps["bq"]; bk_d = aps["bk"]; bv_d = aps["bv"]
    out_d = aps["out"]
    m01_d = aps.get("mask01")

    from contextlib import ExitStack
    stack = ExitStack()
    consts = stack.enter_context(tc.tile_pool(name="consts", bufs=1))
    xpool = stack.enter_context(tc.tile_pool(name="xpool", bufs=1))
    projpool = stack.enter_context(tc.tile_pool(name="projpool", bufs=1))

    # --- constants ---
    identity = consts.tile([128, 128], BF16)
    make_identity(nc, identity)

    wo_sb = consts.tile([128, PAIRS, C_], BF16)
    nc.sync.dma_start(out=wo_sb, in_=wo_d.rearrange("(j p) o -> p j o", p=128))

    # x^T in bf16 (projections) and fp8 (sim), both host-precast
    xbf_sb = xpool.tile([128, CT, S_], BF16)
    xh8_sb = xpool.tile([128, CT, S_], FP8)
    for c in range(CT):
        nc.sync.dma_start(out=xbf_sb[:, c, :], in_=xbf_d[c * 128:(c + 1) * 128, :])
    for c in range(CT):
        nc.sync.dma_start(out=xh8_sb[:, c, :], in_=xh8_d[c * 128:(c + 1) * 128, :])

    # projections
    qT_sb = projpool.tile([128, PAIRS, S_], BF16)
    kT_sb = projpool.tile([128, PAIRS, S_], BF16)
    v2_sb = projpool.tile([128, HPC_, NKT, VW], BF16)

    with tc.tile_pool(name="wpool", bufs=1) as wpool, \
         tc.tile_pool(name="vstage", bufs=1) as vstage, \
         tc.tile_pool(name="ph1psum", bufs=2, space="PSUM") as prj_ps, \
         tc.tile_pool(name="tppsum", bufs=4, space="PSUM") as tp_ps:
        wq_sb = wpool.tile([128, CT, D2], BF16)
        wk_sb = wpool.tile([128, CT, D2], BF16)
        wv_sb = wpool.tile([128, CT, D2], BF16)
        for w_sb, w_d in ((wq_sb, wq_d), (wk_sb, wk_d), (wv_sb, wv_d)):
            nc.sync.dma_start(out=w_sb, in_=w_d.rearrange("(t p) m -> p t m", p=128))
        bq_sb = wpool.tile([128, PAIRS, 1], F32)
        bk_sb = wpool.tile([128, PAIRS, 1], F32)
        bv_sb = wpool.tile([128, PAIRS, 1], F32)
        for b_sb, b_d in ((bq_sb, bq_d), (bk_sb, bk_d), (bv_sb, bv_d)):
            nc.sync.dma_start(
                out=b_sb, in_=b_d.rearrange("(j p) one -> p j one", p=128))

        vT_sb = vstage.tile([128, PAIRS, S_], BF16)
        for w_sb, b_sb, dest in (
            (wq_sb, bq_sb, qT_sb),
            (wk_sb, bk_sb, kT_sb),
            (wv_sb, bv_sb, vT_sb),
        ):
            for nb in range(NPB):
                pss = [prj_ps.tile([128, PB], F32, tag=f"prj{j}",
                                   name=f"prj_{dest.tensor.name}_{nb}_{j}")
                       for j in range(PAIRS)]
                for c in range(CT):
                    for j in range(PAIRS):
                        nc.tensor.matmul(
                            pss[j],
                            w_sb[:, c, j * 128:(j + 1) * 128],
                            xbf_sb[:, c, nb * PB:(nb + 1) * PB],
                            start=(c == 0),
                            stop=(c == CT - 1),
                        )
                for j in range(PAIRS):
                    nc.vector.tensor_scalar_add(
                        dest[:, j, nb * PB:(nb + 1) * PB], pss[j], b_sb[:, j, :]
                    )

        if debug_taps:
            nc.sync.dma_start(out=aps["dbg_qT"], in_=qT_sb)
            nc.sync.dma_start(out=aps["dbg_kT"], in_=kT_sb)

        # V: PE-transpose vT (bf16) -> [keys, d] layout, 2 heads per tile
        for j in range(PAIRS):
            for t in range(NKT):
                tp = tp_ps.tile([128, 128], BF16, tag="tp")
                nc.tensor.transpose(tp, vT_sb[:, j, t * 128:(t + 1) * 128], identity)
                nc.vector.tensor_copy(
                    v2_sb[:, 2 * j:2 * j + 2, t, 0:HEAD_DIM],
                    tp.rearrange("p (h d) -> p h d", h=2),
                )
        nc.vector.memset(v2_sb[:, :, :, HEAD_DIM:HEAD_DIM + 1], 1.0)
        if debug_taps:
            nc.sync.dma_start(out=aps["dbg_v2"],
                              in_=v2_sb[:, :, :, 0:HEAD_DIM + 1])

    # --- main loop (phase 2) ---
    ctxT2_sb = projpool.tile([128, PAIRS, S_], BF16)
    ptpool = stack.enter_context(tc.tile_pool(name="ptpool", bufs=16))
    espool = stack.enter_context(tc.tile_pool(name="espool", bufs=4))
    enpool = stack.enter_context(tc.tile_pool(name="enpool", bufs=3))
    smallpool = stack.enter_context(tc.tile_pool(name="smallpool", bufs=2))
    mpool = (stack.enter_context(tc.tile_pool(name="mpool", bufs=2))
             if m01_d is not None else None)

    with tc.tile_pool(name="simpsum", bufs=1, space="PSUM") as simp, \
         tc.tile_pool(name="scpsum", bufs=3, space="PSUM") as scp, \
         tc.tile_pool(name="ctxpsum", bufs=1, space="PSUM") as ctxp:

        def emit_ctx(ctx_ps, kt, pts):
            for h in range(HPC_):
                nc.tensor.matmul(
                    ctx_ps[h],
                    v2_sb[:, h, kt, 0:HEAD_DIM + 1],
                    pts[h],
                    start=(kt == 0),
                    stop=(kt == NKT - 1),
                    skip_group_check=True,
                )

        def emit_division_head(qb, ctx_ps, h):
            j, hi = divmod(h, 2)
            r0 = smallpool.tile([1, QB], F32, tag=f"r0{h % 2}",
                                name=f"r0_{qb}_{h}")
            if debug_taps and qb == 0 and h == 0:
                dbg_s = smallpool.tile([1, QB], F32, tag="dbgs")
                nc.vector.tensor_copy(dbg_s, ctx_ps[h][HEAD_DIM:HEAD_DIM + 1, :])
                nc.sync.dma_start(out=aps["dbg_sums0"], in_=dbg_s)
                # A/B: approx reciprocal from SBUF input
                dbg_r1 = smallpool.tile([1, QB], F32, tag="dbgr1")
                nc.vector.reciprocal_approx_fast(dbg_r1, dbg_s)
                nc.sync.dma_start(out=aps["dbg_r1"], in_=dbg_r1)
            nc.vector.reciprocal(r0, ctx_ps[h][HEAD_DIM:HEAD_DIM + 1, :])
            if debug_taps and qb == 0 and h == 0:
                nc.sync.dma_start(out=aps["dbg_r0"], in_=r0)
            rb = smallpool.tile([HEAD_DIM, QB], F32, tag="rb")
            nc.gpsimd.partition_broadcast(rb, r0, channels=HEAD_DIM)
            nc.vector.tensor_mul(
                ctxT2_sb[hi * 64:hi * 64 + 64, j, qb * QB:(qb + 1) * QB],
                ctx_ps[h][0:HEAD_DIM, :],
                rb,
            )

        def emit_division(qb, ctx_ps):
            for h in range(HPC_):
                emit_division_head(qb, ctx_ps, h)

        prev_div = None
        for qb in range(NQB):
            ctx_ps = [ctxp.tile([HEAD_DIM + 1, QB], F32, tag=f"ctx{h}",
                                name=f"ctx_{qb}_{h}")
                      for h in range(HPC_)]
            pending = []
            for kt in range(NKT):
                if prev_div is not None and kt >= 2 and (kt - 2) % 3 == 0:
                    h = (kt - 2) // 3
                    if h < HPC_:
                        emit_division_head(prev_div[0], prev_div[1], h)
                        if h == HPC_ - 1:
                            prev_div = None
                # sim via fp8 DoubleRow: 2 hidden-chunks per pass
                sp = simp.tile([128, QB], F32, tag="sim")
                for c2 in range(CT // 2):
                    nc.tensor.matmul(
                        sp,
                        xh8_sb[:, 2 * c2:2 * c2 + 2, kt * 128:(kt + 1) * 128],
                        xh8_sb[:, 2 * c2:2 * c2 + 2, qb * QB:(qb + 1) * QB],
                        start=(c2 == 0),
                        stop=(c2 == CT // 2 - 1),
                        perf_mode=DR,
                    )
                en = enpool.tile([128, QB], BF16, tag="en")
                nc.scalar.activation(out=en, in_=sp, func=ACT_EXP,
                                     scale=neg_gamma_scale)
                if debug_taps and qb == 0 and kt == 0:
                    nc.sync.dma_start(out=aps["dbg_en0"], in_=en)
                if m01_d is not None:
                    m_sb = mpool.tile([128, QB], BF16, tag="msk")
                    nc.sync.dma_start(
                        out=m_sb,
                        in_=m01_d[kt * 128:(kt + 1) * 128, qb * QB:(qb + 1) * QB],
                    )
                    nc.vector.tensor_mul(en, en, m_sb)
                pts = []
                for h in range(HPC_):
                    j, hi = divmod(h, 2)
                    pr = slice(hi * 64, hi * 64 + 64)
                    sc_t = scp.tile([128, QB], F32, tag="sc")
                    nc.tensor.matmul(
                        sc_t,
                        kT_sb[pr, j, kt * 128:(kt + 1) * 128],
                        qT_sb[pr, j, qb * QB:(qb + 1) * QB],
                        start=True,
                        stop=True,
                    )
                    es = espool.tile([128, QB], BF16, tag="es")
                    nc.scalar.activation(out=es, in_=sc_t, func=ACT_EXP)
                    pt = ptpool.tile([128, QB], BF16, tag="pt")
                    nc.vector.tensor_mul(pt, es, en)
                    if debug_taps and qb == 0 and kt == 0 and h == 0:
                        nc.sync.dma_start(out=aps["dbg_pt0"], in_=pt)
                    pts.append(pt)
                pending.append((kt, pts))
                if len(pending) > LAG:
                    k0, p0 = pending.pop(0)
                    emit_ctx(ctx_ps, k0, p0)
            for k0, p0 in pending:
                emit_ctx(ctx_ps, k0, p0)
            if prev_div is not None:
                done = max(0, (NKT - 1 - 2) // 3 + 1) if NKT > 2 else 0
                for h in range(min(done, HPC_), HPC_):
                    emit_division_head(prev_div[0], prev_div[1], h)
                prev_div = None
            prev_div = (qb, ctx_ps)
        emit_division(*prev_div)

    if debug_taps:
        nc.sync.dma_start(out=aps["dbg_ctxT2"], in_=ctxT2_sb)

    # --- out-projection (phase 3) ---
    with tc.tile_pool(name="outpsum", bufs=4, space="PSUM") as outp, \
         tc.tile_pool(name="outstg", bufs=4) as outstg:
        for qt in range(S_ // 128):
            for ob in range(NOB):
                op = outp.tile([128, OB_W], F32, tag="op")
                for j in range(PAIRS):
                    nc.tensor.matmul(
                        op,
                        ctxT2_sb[:, j, qt * 128:(qt + 1) * 128],
                        wo_sb[:, j, ob * OB_W:(ob + 1) * OB_W],
                        start=(j == 0),
                        stop=(j == PAIRS - 1),
                    )
                ostg = outstg.tile([128, OB_W], F32, tag="ostg")
                if (qt + ob) % 2 == 0:
                    nc.scalar.activation(out=ostg, in_=op, func=ACT_COPY)
                else:
                    nc.vector.tensor_copy(ostg, op)
                nc.sync.dma_start(
                    out=out_d[qt * 128:(qt + 1) * 128, ob * OB_W:(ob + 1) * OB_W],
                    in_=ostg,
                )

    stack.close()


def build_nc(*, S_=S, C_=HIDDEN, HPC_=HPC, QB=512, with_mask=False,
             enable_asserts=False, debug_taps=False):
    nc = bacc.Bacc(
        "TRN2", target_bir_lowering=False, debug=False,
        enable_asserts=enable_asserts,
    )
    D2 = HPC_ * HEAD_DIM
    PAIRS = HPC_ // 2
    NKT = S_ // 128
    aps = {}
    aps["xbf"] = nc.dram_tensor("xbf", [C_, S_], BF16, kind="ExternalInput").ap()
    aps["xh8"] = nc.dram_tensor("xh8", [C_, S_], FP8, kind="ExternalInput").ap()
    for n in ("wq", "wk", "wv"):
        aps[n] = nc.dram_tensor(n, [C_, D2], BF16, kind="ExternalInput").ap()
    aps["wo"] = nc.dram_tensor("wo", [D2, C_], BF16, kind="ExternalInput").ap()
    for n in ("bq", "bk", "bv"):
        aps[n] = nc.dram_tensor(n, [D2, 1], F32, kind="ExternalInput").ap()
    if with_mask:
        aps["mask01"] = nc.dram_tensor(
            "mask01", [S_, S_], BF16, kind="ExternalInput").ap()
    aps["out"] = nc.dram_tensor("out", [S_, C_], F32, kind="ExternalOutput").ap()
    if debug_taps:
        aps["dbg_qT"] = nc.dram_tensor(
            "dbg_qT", [128, PAIRS, S_], BF16, kind="ExternalOutput").ap()
        aps["dbg_kT"] = nc.dram_tensor(
            "dbg_kT", [128, PAIRS, S_], BF16, kind="ExternalOutput").ap()
        aps["dbg_v2"] = nc.dram_tensor(
            "dbg_v2", [128, HPC_, NKT, HEAD_DIM + 1], BF16,
            kind="ExternalOutput").ap()
        aps["dbg_ctxT2"] = nc.dram_tensor(
            "dbg_ctxT2", [128, PAIRS, S_], BF16, kind="ExternalOutput").ap()
        aps["dbg_en0"] = nc.dram_tensor(
            "dbg_en0", [128, QB], BF16, kind="ExternalOutput").ap()
        aps["dbg_pt0"] = nc.dram_tensor(
            "dbg_pt0", [128, QB], BF16, kind="ExternalOutput").ap()
        aps["dbg_sums0"] = nc.dram_tensor(
            "dbg_sums0", [1, QB], F32, kind="ExternalOutput").ap()
        aps["dbg_r0"] = nc.dram_tensor(
            "dbg_r0", [1, QB], F32, kind="ExternalOutput").ap()
        aps["dbg_r1"] = nc.dram_tensor(
            "dbg_r1", [1, QB], F32, kind="ExternalOutput").ap()

    with tile.TileContext(nc) as tc:
        emit_kernel(tc, aps, S_=S_, C_=C_, HPC_=HPC_, QB=QB,
                    with_mask=with_mask, debug_taps=debug_taps)
    nc.compile()
    return nc


def host_prepare(x, attn_mask, Wq, bq, Wk, bk, Wv, bv, Wo, bo, *,
                 S_=S, C_=HIDDEN, HPC_=HPC, n_cores=N_CORES):
    """Build the per-core input maps. Returns (in_maps, with_mask)."""
    bf = ml_dtypes.bfloat16
    f8 = ml_dtypes.float8_e4m3fn
    x = np.asarray(x, np.float32)
    B_ = x.shape[0]
    groups = n_cores // B_
    Wq = np.asarray(Wq, np.float32); Wk = np.asarray(Wk, np.float32)
    Wv = np.asarray(Wv, np.float32); Wo = np.asarray(Wo, np.float32)
    bq = np.asarray(bq, np.float32); bk = np.asarray(bk, np.float32)
    bv = np.asarray(bv, np.float32)

    inv_sqrt_d = 1.0 / math.sqrt(HEAD_DIM)
    WqT = np.ascontiguousarray((Wq * inv_sqrt_d).T).astype(bf)  # [C, C]
    WkT = np.ascontiguousarray(Wk.T).astype(bf)
    WvT = np.ascontiguousarray(Wv.T).astype(bf)
    WoT = np.ascontiguousarray(Wo.T).astype(bf)                 # [C(c), C(o)]
    bq = bq * inv_sqrt_d

    mask = np.asarray(attn_mask)
    with_mask = bool(mask.any())
    mask01 = None
    if with_mask:
        # reference: where(mask, -inf) -> multiplicative 0/1 on P
        mask01 = np.where(mask, 0.0, 1.0).astype(bf)
        mask01 = np.ascontiguousarray(mask01.T)  # [k, q]

    in_maps = []
    for core in range(n_cores):
        b, g = divmod(core, groups)
        xb = x[b]                                   # [S, C]
        xT = np.ascontiguousarray(xb.T)             # [C, S]
        norms = np.linalg.norm(xb, axis=1)          # [S]
        scale = (FP8_SCALE / np.maximum(norms, 1e-12)).astype(np.float32)
        xh8 = (xT * scale[None, :]).astype(f8)
        ch = slice(g * HPC_ * HEAD_DIM, (g + 1) * HPC_ * HEAD_DIM)
        m = {
            "xbf": xT.astype(bf),
            "xh8": xh8,
            "wq": np.ascontiguousarray(WqT[:, ch]),
            "wk": np.ascontiguousarray(WkT[:, ch]),
            "wv": np.ascontiguousarray(WvT[:, ch]),
            "wo": np.ascontiguousarray(WoT[ch, :]),
            "bq": np.ascontiguousarray(bq[ch]).reshape(-1, 1),
            "bk": np.ascontiguousarray(bk[ch]).reshape(-1, 1),
            "bv": np.ascontiguousarray(bv[ch]).reshape(-1, 1),
        }
        if with_mask:
            m["mask01"] = mask01
        in_maps.append(m)
    return in_maps, with_mask


_NC_CACHE = {}


def _get_nc(with_mask):
    key = with_mask
    if key not in _NC_CACHE:
        _NC_CACHE[key] = build_nc(with_mask=with_mask)
    return _NC_CACHE[key]


LAST_RESULTS = None


def kernel(**inputs):
    global LAST_RESULTS
    in_maps, with_mask = host_prepare(
        inputs["x"], inputs["attn_mask"],
        inputs["Wq"], inputs["bq"], inputs["Wk"], inputs["bk"],
        inputs["Wv"], inputs["bv"], inputs["Wo"], inputs["bo"],
    )
    nc = _get_nc(with_mask)
    res = run_bass_kernel_spmd(nc, in_maps, core_ids=list(range(N_CORES)))
    LAST_RESULTS = res
    bo = np.asarray(inputs["bo"], np.float32)
    out = np.zeros((B, S, HIDDEN), np.float32)
    groups = N_CORES // B
    for core in range(N_CORES):
        b = core // groups
        out[b] += res.results[core]["out"]
    out += bo[None, None, :]
    return out


# revision 17
# speedup vs baseline: 1.5087x; 1.1338x over previous
"""DiversityAttention on 8 TRN2 NeuronCores (Bass/Tile), bf16/fp8 edition.

Sharding: data-parallel over batch (B=2) x tensor-parallel over heads
(16 heads -> 4 groups of 4). core = (b, g), b = core // 4, g = core % 4.
Each core computes full attention for its 4 heads over its batch and a
partial out-projection [S, HIDDEN]; the host sums the 4 partials per
batch and adds bo.

Device-side formulation (keys-on-partitions / "S^T" orientation):
  qT = (Wq/sqrt(dh) @ x^T + bq')  [64h, S]  bf16
  kT = (Wk @ x^T + bk)            [64h, S]  bf16
  vT = (Wv @ x^T + bv) then PE-transposed to V [S, 64h] bf16 (+ ones col)
  xh8 = fp8(64 * x^T / max(||x||, eps))  (host-precomputed)
  per (qb, kt):
     sim_psum[k,q] = xh8^T xh8   (fp8 DoubleRow matmuls, 2 chunks/pass)
     En = exp(-gamma/4096 * sim_psum)            (ACT, bf16)
     per head: sc_psum[k,q] = kT^T qT            (bf16 matmul)
               Es = exp(sc_psum)                 (ACT, bf16)
               P  = Es * En                      (DVE 2x bf16)
  ctx^T[d,q] (+sums row) = sum_k [V|1]^T P       (bf16 matmul, PSUM accum)
  ctx normalized via reciprocal_approx_fast on an SBUF copy of sums
  out[q,o] partial = ctxT^T @ WoT (bf16)  -> DMA to DRAM (f32)
"""

import math
import os
import sys

import numpy as np

for _p in ("/opt/trn_rl_repo",):
    if _p not in sys.path and os.path.isdir(_p):
        sys.path.insert(0, _p)

os.environ.setdefault("MYCRO_LOCAL_CACHE", "1")

import ml_dtypes

import concourse.bass as bass
import concourse.tile as tile
from concourse import bacc, mybir
from concourse.bass_utils import run_bass_kernel_spmd
from concourse.masks import make_identity


def _install_ntff_hook():
    """Provide antenv.axon_hooks (NTFF profiling registry) if the image
    lacks it, mirroring trn_agent_boot's ctypes hook. No-op on failure."""
    try:
        import antenv.axon_hooks  # noqa: F401
        return
    except ImportError:
        pass
    try:
        import contextlib
        import ctypes
        import types

        so_path = "/opt/axon/libaxon_pjrt.so"
        if not os.path.exists(so_path):
            return
        lib = ctypes.CDLL(so_path)
        if not hasattr(lib, "axon_start_nrt_profile"):
            return
        lib.axon_start_nrt_profile.argtypes = [
            ctypes.POINTER(ctypes.c_int64), ctypes.c_size_t]
        lib.axon_start_nrt_profile.restype = ctypes.c_int64
        lib.axon_stop_nrt_profile.argtypes = [ctypes.c_char_p]
        lib.axon_stop_nrt_profile.restype = ctypes.c_int64

        @contextlib.contextmanager
        def _hook(output_dir, device_ids):
            import jax
            jax.devices()
            if device_ids:
                ids = (ctypes.c_int64 * len(device_ids))(*device_ids)
                rc = lib.axon_start_nrt_profile(ids, len(device_ids))
            else:
                rc = lib.axon_start_nrt_profile(None, 0)
            if rc != 0:
                raise RuntimeError(f"axon_start_nrt_profile rc={rc}")
            try:
                yield
            finally:
                n = lib.axon_stop_nrt_profile(str(output_dir).encode())
                print(f"ntff profile: {n} file(s) -> {output_dir}",
                      file=sys.stderr)

        mod = types.ModuleType("antenv.axon_hooks")
        _state = {"hook": _hook}
        mod.set_axon_ntff_profile_hook = lambda h: _state.__setitem__("hook", h)
        mod.get_axon_ntff_profile_hook = lambda: _state["hook"]
        sys.modules["antenv.axon_hooks"] = mod
        import antenv
        antenv.axon_hooks = mod
    except Exception:
        pass


_install_ntff_hook()

F32 = mybir.dt.float32
BF16 = mybir.dt.bfloat16
FP8 = mybir.dt.float8e4
ALU = mybir.AluOpType
ACT_EXP = mybir.ActivationFunctionType.Exp
ACT_COPY = mybir.ActivationFunctionType.Copy
DR = mybir.MatmulPerfMode.DoubleRow

# Problem constants (hardcoded per contract).
HIDDEN = 1024
HEADS = 16
HEAD_DIM = 64
GAMMA = 0.5
B, S = 2, 2048
N_CORES = 8
GROUPS = N_CORES // B  # head groups per batch
HPC = HEADS // GROUPS  # heads per core
LAG = 2  # kt software-pipeline lag between P and ctx matmul
FP8_SCALE = 64.0  # host scales normalized x by this before fp8 cast
VW = 66  # v2 padded row width (64 dims + ones col + pad)
SIM_DR = True  # fp8 DoubleRow for the sim matmuls


def emit_kernel(tc, aps, *, S_, C_, HPC_, QB, with_mask, debug_taps=False):
    """Emit the per-core kernel. aps: dict of dram APs."""
    nc = tc.nc
    CT = C_ // 128          # contraction chunks over hidden
    PAIRS = HPC_ // 2       # head pairs (128-channel chunks)
    NKT = S_ // 128         # key tiles
    NQB = S_ // QB          # query blocks
    PB = min(512, S_)       # projection free-block width
    NPB = S_ // PB
    OB_W = min(512, C_)     # out-projection free-block width
    NOB = C_ // OB_W
    D2 = HPC_ * HEAD_DIM
    neg_gamma_scale = -GAMMA / (FP8_SCALE * FP8_SCALE)

    xbf_d = aps["xbf"]; xh8_d = aps["xh8"]
    wq_d = aps["wq"]; wk_d = aps["wk"]; wv_d = aps["wv"]; wo_d = aps["wo"]
    bq_d = aps["bq"]; bk_d = aps["bk"]; bv_d = aps["bv"]
    out_d = aps["out"]
    m01_d = aps.get("mask01")

    from contextlib import ExitStack
    stack = ExitStack()
    consts = stack.enter_context(tc.tile_pool(name="consts", bufs=1))
    xpool = stack.enter_context(tc.tile_pool(name="xpool", bufs=1))
    projpool = stack.enter_context(tc.tile_pool(name="projpool", bufs=1))

    identity = consts.tile([128, 128], BF16)
    make_identity(nc, identity)

    wo_sb = consts.tile([128, PAIRS, C_], BF16)
    nc.sync.dma_start(out=wo_sb, in_=wo_d.rearrange("(j p) o -> p j o", p=128))

    # x^T in bf16 (projections) and fp8 (sim), both host-precast
    xbf_sb = xpool.tile([128, CT, S_], BF16)
    xh8_sb = xpool.tile([128, CT, S_], FP8)
    for c in range(CT):
        nc.sync.dma_start(out=xbf_sb[:, c, :], in_=xbf_d[c * 128:(c + 1) * 128, :])
    for c in range(CT):
        nc.sync.dma_start(out=xh8_sb[:, c, :], in_=xh8_d[c * 128:(c + 1) * 128, :])

    # projections
    qT_sb = projpool.tile([128, PAIRS, S_], BF16)
    kT_sb = projpool.tile([128, PAIRS, S_], BF16)
    v2_sb = projpool.tile([128, HPC_, NKT, VW], BF16)

    with tc.tile_pool(name="wpool", bufs=1) as wpool, \
         tc.tile_pool(name="vstage", bufs=1) as vstage, \
         tc.tile_pool(name="ph1psum", bufs=2, space="PSUM") as prj_ps, \
         tc.tile_pool(name="tppsum", bufs=4, space="PSUM") as tp_ps:
        wq_sb = wpool.tile([128, CT, D2], BF16)
        wk_sb = wpool.tile([128, CT, D2], BF16)
        wv_sb = wpool.tile([128, CT, D2], BF16)
        for w_sb, w_d in ((wq_sb, wq_d), (wk_sb, wk_d), (wv_sb, wv_d)):
            nc.sync.dma_start(out=w_sb, in_=w_d.rearrange("(t p) m -> p t m", p=128))
        bq_sb = wpool.tile([128, PAIRS, 1], F32)
        bk_sb = wpool.tile([128, PAIRS, 1], F32)
        bv_sb = wpool.tile([128, PAIRS, 1], F32)
        for b_sb, b_d in ((bq_sb, bq_d), (bk_sb, bk_d), (bv_sb, bv_d)):
            nc.sync.dma_start(
                out=b_sb, in_=b_d.rearrange("(j p) one -> p j one", p=128))

        vT_sb = vstage.tile([128, PAIRS, S_], BF16)
        for w_sb, b_sb, dest in (
            (wq_sb, bq_sb, qT_sb),
            (wk_sb, bk_sb, kT_sb),
            (wv_sb, bv_sb, vT_sb),
        ):
            for nb in range(NPB):
                pss = [prj_ps.tile([128, PB], F32, tag=f"prj{j}",
                                   name=f"prj_{dest.tensor.name}_{nb}_{j}")
                       for j in range(PAIRS)]
                for c in range(CT):
                    for j in range(PAIRS):
                        nc.tensor.matmul(
                            pss[j],
                            w_sb[:, c, j * 128:(j + 1) * 128],
                            xbf_sb[:, c, nb * PB:(nb + 1) * PB],
                            start=(c == 0),
                            stop=(c == CT - 1),
                        )
                for j in range(PAIRS):
                    nc.vector.tensor_scalar_add(
                        dest[:, j, nb * PB:(nb + 1) * PB], pss[j], b_sb[:, j, :]
                    )

        if debug_taps:
            nc.sync.dma_start(out=aps["dbg_qT"], in_=qT_sb)
            nc.sync.dma_start(out=aps["dbg_kT"], in_=kT_sb)

        # V: PE-transpose vT (bf16) -> [keys, d] layout, 2 heads per tile
        for j in range(PAIRS):
            for t in range(NKT):
                tp = tp_ps.tile([128, 128], BF16, tag="tp")
                nc.tensor.transpose(tp, vT_sb[:, j, t * 128:(t + 1) * 128], identity)
                nc.vector.tensor_copy(
                    v2_sb[:, 2 * j:2 * j + 2, t, 0:HEAD_DIM],
                    tp.rearrange("p (h d) -> p h d", h=2),
                )
        nc.vector.memset(v2_sb[:, :, :, HEAD_DIM:HEAD_DIM + 1], 1.0)
        if debug_taps:
            nc.sync.dma_start(out=aps["dbg_v2"],
                              in_=v2_sb[:, :, :, 0:HEAD_DIM + 1])

    # --- main loop (phase 2) ---
    ctxT2_sb = projpool.tile([128, PAIRS, S_], BF16)
    ptpool = stack.enter_context(tc.tile_pool(name="ptpool", bufs=16))
    espool = stack.enter_context(tc.tile_pool(name="espool", bufs=4))
    enpool = stack.enter_context(tc.tile_pool(name="enpool", bufs=3))
    smallpool = stack.enter_context(tc.tile_pool(name="smallpool", bufs=2))
    mpool = (stack.enter_context(tc.tile_pool(name="mpool", bufs=2))
             if m01_d is not None else None)

    with tc.tile_pool(name="simpsum", bufs=1, space="PSUM") as simp, \
         tc.tile_pool(name="scpsum", bufs=3, space="PSUM") as scp, \
         tc.tile_pool(name="ctxpsum", bufs=1, space="PSUM") as ctxp:

        def emit_ctx(ctx_ps, kt, pts):
            for h in range(HPC_):
                nc.tensor.matmul(
                    ctx_ps[h],
                    v2_sb[:, h, kt, 0:HEAD_DIM + 1],
                    pts[h],
                    start=(kt == 0),
                    stop=(kt == NKT - 1),
                    skip_group_check=True,
                )

        def emit_division_head(qb, ctx_ps, h):
            j, hi = divmod(h, 2)
            s_sb = smallpool.tile([1, QB], F32, tag=f"ssb{h % 2}",
                                  name=f"ssb_{qb}_{h}")
            nc.vector.tensor_copy(s_sb, ctx_ps[h][HEAD_DIM:HEAD_DIM + 1, :])
            r0 = smallpool.tile([1, QB], F32, tag=f"r0{h % 2}",
                                name=f"r0_{qb}_{h}")
            nc.vector.reciprocal_approx_fast(r0, s_sb)
            if debug_taps and qb == 0 and h == 0:
                nc.sync.dma_start(out=aps["dbg_r0"], in_=r0)
            rb = smallpool.tile([HEAD_DIM, QB], F32, tag="rb")
            nc.gpsimd.partition_broadcast(rb, r0, channels=HEAD_DIM)
            nc.vector.tensor_mul(
                ctxT2_sb[hi * 64:hi * 64 + 64, j, qb * QB:(qb + 1) * QB],
                ctx_ps[h][0:HEAD_DIM, :],
                rb,
            )

        def emit_division(qb, ctx_ps):
            for h in range(HPC_):
                emit_division_head(qb, ctx_ps, h)

        prev_div = None
        for qb in range(NQB):
            ctx_ps = [ctxp.tile([HEAD_DIM + 1, QB], F32, tag=f"ctx{h}",
                                name=f"ctx_{qb}_{h}")
                      for h in range(HPC_)]
            pending = []
            for kt in range(NKT):
                if prev_div is not None and kt >= 2 and (kt - 2) % 3 == 0:
                    h = (kt - 2) // 3
                    if h < HPC_:
                        emit_division_head(prev_div[0], prev_div[1], h)
                        if h == HPC_ - 1:
                            prev_div = None
                # sim via fp8 DoubleRow: 2 hidden-chunks per pass
                sp = simp.tile([128, QB], F32, tag="sim")
                if SIM_DR:
                    for c2 in range(CT // 2):
                        nc.tensor.matmul(
                            sp,
                            xh8_sb[:, 2 * c2:2 * c2 + 2, kt * 128:(kt + 1) * 128],
                            xh8_sb[:, 2 * c2:2 * c2 + 2, qb * QB:(qb + 1) * QB],
                            start=(c2 == 0),
                            stop=(c2 == CT // 2 - 1),
                            perf_mode=DR,
                        )
                else:
                    for c in range(CT):
                        nc.tensor.matmul(
                            sp,
                            xh8_sb[:, c, kt * 128:(kt + 1) * 128],
                            xh8_sb[:, c, qb * QB:(qb + 1) * QB],
                            start=(c == 0),
                            stop=(c == CT - 1),
                        )
                en = enpool.tile([128, QB], BF16, tag="en")
                nc.scalar.activation(out=en, in_=sp, func=ACT_EXP,
                                     scale=neg_gamma_scale)
                if debug_taps and qb == 0 and kt == 0:
                    nc.sync.dma_start(out=aps["dbg_en0"], in_=en)
                if m01_d is not None:
                    m_sb = mpool.tile([128, QB], BF16, tag="msk")
                    nc.sync.dma_start(
                        out=m_sb,
                        in_=m01_d[kt * 128:(kt + 1) * 128, qb * QB:(qb + 1) * QB],
                    )
                    nc.vector.tensor_mul(en, en, m_sb)
                pts = []
                for h in range(HPC_):
                    j, hi = divmod(h, 2)
                    pr = slice(hi * 64, hi * 64 + 64)
                    sc_t = scp.tile([128, QB], F32, tag="sc")
                    nc.tensor.matmul(
                        sc_t,
                        kT_sb[pr, j, kt * 128:(kt + 1) * 128],
                        qT_sb[pr, j, qb * QB:(qb + 1) * QB],
                        start=True,
                        stop=True,
                    )
                    es = espool.tile([128, QB], BF16, tag="es")
                    nc.scalar.activation(out=es, in_=sc_t, func=ACT_EXP)
                    pt = ptpool.tile([128, QB], BF16, tag="pt")
                    nc.vector.tensor_mul(pt, es, en)
                    if debug_taps and qb == 0 and kt == 0 and h == 0:
                        nc.sync.dma_start(out=aps["dbg_pt0"], in_=pt)
                    pts.append(pt)
                pending.append((kt, pts))
                if len(pending) > LAG:
                    k0, p0 = pending.pop(0)
                    emit_ctx(ctx_ps, k0, p0)
            for k0, p0 in pending:
                emit_ctx(ctx_ps, k0, p0)
            if prev_div is not None:
                done = max(0, (NKT - 1 - 2) // 3 + 1) if NKT > 2 else 0
                for h in range(min(done, HPC_), HPC_):
                    emit_division_head(prev_div[0], prev_div[1], h)
                prev_div = None
            prev_div = (qb, ctx_ps)
        emit_division(*prev_div)

    if debug_taps:
        nc.sync.dma_start(out=aps["dbg_ctxT2"], in_=ctxT2_sb)

    # --- out-projection (phase 3) ---
    with tc.tile_pool(name="outpsum", bufs=4, space="PSUM") as outp, \
         tc.tile_pool(name="outstg", bufs=4) as outstg:
        for qt in range(S_ // 128):
            for ob in range(NOB):
                op = outp.tile([128, OB_W], F32, tag="op")
                for j in range(PAIRS):
                    nc.tensor.matmul(
                        op,
                        ctxT2_sb[:, j, qt * 128:(qt + 1) * 128],
                        wo_sb[:, j, ob * OB_W:(ob + 1) * OB_W],
                        start=(j == 0),
                        stop=(j == PAIRS - 1),
                    )
                ostg = outstg.tile([128, OB_W], F32, tag="ostg")
                nc.vector.tensor_copy(ostg, op)
                nc.sync.dma_start(
                    out=out_d[qt * 128:(qt + 1) * 128, ob * OB_W:(ob + 1) * OB_W],
                    in_=ostg,
                )

    stack.close()


def build_nc(*, S_=S, C_=HIDDEN, HPC_=HPC, QB=512, with_mask=False,
             enable_asserts=False, debug_taps=False):
    nc = bacc.Bacc(
        "TRN2", target_bir_lowering=False, debug=False,
        enable_asserts=enable_asserts,
    )
    D2 = HPC_ * HEAD_DIM
    PAIRS = HPC_ // 2
    NKT = S_ // 128
    aps = {}
    aps["xbf"] = nc.dram_tensor("xbf", [C_, S_], BF16, kind="ExternalInput").ap()
    aps["xh8"] = nc.dram_tensor("xh8", [C_, S_], FP8, kind="ExternalInput").ap()
    for n in ("wq", "wk", "wv"):
        aps[n] = nc.dram_tensor(n, [C_, D2], BF16, kind="ExternalInput").ap()
    aps["wo"] = nc.dram_tensor("wo", [D2, C_], BF16, kind="ExternalInput").ap()
    for n in ("bq", "bk", "bv"):
        aps[n] = nc.dram_tensor(n, [D2, 1], F32, kind="ExternalInput").ap()
    if with_mask:
        aps["mask01"] = nc.dram_tensor(
            "mask01", [S_, S_], BF16, kind="ExternalInput").ap()
    aps["out"] = nc.dram_tensor("out", [S_, C_], F32, kind="ExternalOutput").ap()
    if debug_taps:
        aps["dbg_qT"] = nc.dram_tensor(
            "dbg_qT", [128, PAIRS, S_], BF16, kind="ExternalOutput").ap()
        aps["dbg_kT"] = nc.dram_tensor(
            "dbg_kT", [128, PAIRS, S_], BF16, kind="ExternalOutput").ap()
        aps["dbg_v2"] = nc.dram_tensor(
            "dbg_v2", [128, HPC_, NKT, HEAD_DIM + 1], BF16,
            kind="ExternalOutput").ap()
        aps["dbg_ctxT2"] = nc.dram_tensor(
            "dbg_ctxT2", [128, PAIRS, S_], BF16, kind="ExternalOutput").ap()
        aps["dbg_en0"] = nc.dram_tensor(
            "dbg_en0", [128, QB], BF16, kind="ExternalOutput").ap()
        aps["dbg_pt0"] = nc.dram_tensor(
            "dbg_pt0", [128, QB], BF16, kind="ExternalOutput").ap()
        aps["dbg_r0"] = nc.dram_tensor(
            "dbg_r0", [1, QB], F32, kind="ExternalOutput").ap()

    with tile.TileContext(nc) as tc:
        emit_kernel(tc, aps, S_=S_, C_=C_, HPC_=HPC_, QB=QB,
                    with_mask=with_mask, debug_taps=debug_taps)
    nc.compile()
    return nc


def host_prepare(x, attn_mask, Wq, bq, Wk, bk, Wv, bv, Wo, bo, *,
                 S_=S, C_=HIDDEN, HPC_=HPC, n_cores=N_CORES):
    """Build the per-core input maps. Returns (in_maps, with_mask)."""
    bf = ml_dtypes.bfloat16
    f8 = ml_dtypes.float8_e4m3fn
    x = np.asarray(x, np.float32)
    B_ = x.shape[0]
    groups = n_cores // B_
    Wq = np.asarray(Wq, np.float32); Wk = np.asarray(Wk, np.float32)
    Wv = np.asarray(Wv, np.float32); Wo = np.asarray(Wo, np.float32)
    bq = np.asarray(bq, np.float32); bk = np.asarray(bk, np.float32)
    bv = np.asarray(bv, np.float32)

    inv_sqrt_d = 1.0 / math.sqrt(HEAD_DIM)
    WqT = np.ascontiguousarray((Wq * inv_sqrt_d).T).astype(bf)  # [C, C]
    WkT = np.ascontiguousarray(Wk.T).astype(bf)
    WvT = np.ascontiguousarray(Wv.T).astype(bf)
    WoT = np.ascontiguousarray(Wo.T).astype(bf)                 # [C(c), C(o)]
    bq = bq * inv_sqrt_d

    mask = np.asarray(attn_mask)
    with_mask = bool(mask.any())
    mask01 = None
    if with_mask:
        # reference: where(mask, -inf) -> multiplicative 0/1 on P
        mask01 = np.where(mask, 0.0, 1.0).astype(bf)
        mask01 = np.ascontiguousarray(mask01.T)  # [k, q]

    in_maps = []
    for core in range(n_cores):
        b, g = divmod(core, groups)
        xb = x[b]                                   # [S, C]
        xT = np.ascontiguousarray(xb.T)             # [C, S]
        norms = np.linalg.norm(xb, axis=1)          # [S]
        scale = (FP8_SCALE / np.maximum(norms, 1e-12)).astype(np.float32)
        xh8 = (xT * scale[None, :]).astype(f8)
        ch = slice(g * HPC_ * HEAD_DIM, (g + 1) * HPC_ * HEAD_DIM)
        m = {
            "xbf": xT.astype(bf),
            "xh8": xh8,
            "wq": np.ascontiguousarray(WqT[:, ch]),
            "wk": np.ascontiguousarray(WkT[:, ch]),
            "wv": np.ascontiguousarray(WvT[:, ch]),
            "wo": np.ascontiguousarray(WoT[ch, :]),
            "bq": np.ascontiguousarray(bq[ch]).reshape(-1, 1),
            "bk": np.ascontiguousarray(bk[ch]).reshape(-1, 1),
            "bv": np.ascontiguousarray(bv[ch]).reshape(-1, 1),
        }
        if with_mask:
            m["mask01"] = mask01
        in_maps.append(m)
    return in_maps, with_mask


_NC_CACHE = {}


def _get_nc(with_mask):
    key = with_mask
    if key not in _NC_CACHE:
        _NC_CACHE[key] = build_nc(with_mask=with_mask)
    return _NC_CACHE[key]


LAST_RESULTS = None


def kernel(**inputs):
    global LAST_RESULTS
    in_maps, with_mask = host_prepare(
        inputs["x"], inputs["attn_mask"],
        inputs["Wq"], inputs["bq"], inputs["Wk"], inputs["bk"],
        inputs["Wv"], inputs["bv"], inputs["Wo"], inputs["bo"],
    )
    nc = _get_nc(with_mask)
    res = run_bass_kernel_spmd(nc, in_maps, core_ids=list(range(N_CORES)))
    LAST_RESULTS = res
    bo = np.asarray(inputs["bo"], np.float32)
    out = np.zeros((B, S, HIDDEN), np.float32)
    groups = N_CORES // B
    for core in range(N_CORES):
        b = core // groups
        out[b] += res.results[core]["out"]
    out += bo[None, None, :]
    return out


# revision 21
# speedup vs baseline: 1.5794x; 1.0469x over previous
"""DiversityAttention on 8 TRN2 NeuronCores (Bass/Tile), bf16/fp8 edition.

Sharding: data-parallel over batch (B=2) x tensor-parallel over heads
(16 heads -> 4 groups of 4). core = (b, g), b = core // 4, g = core % 4.
Each core computes full attention for its 4 heads over its batch and a
partial out-projection [S, HIDDEN]; the host sums the 4 partials per
batch and adds bo.

Device-side formulation (keys-on-partitions / "S^T" orientation):
  qT = (Wq/sqrt(dh) @ x^T + bq')  [64h, S]  bf16
  kT = (Wk @ x^T + bk)            [64h, S]  bf16
  vT = (Wv @ x^T + bv) then PE-transposed to V [S, 64h] bf16 (+ ones col)
  xh8 = fp8(64 * x^T / max(||x||, eps))  (host-precomputed)
  per (qb, kt):
     sim_psum[k,q] = xh8^T xh8   (fp8 DoubleRow matmuls, 2 chunks/pass)
     En = exp(-gamma/4096 * sim_psum)            (ACT, bf16)
     per head: sc_psum[k,q] = kT^T qT            (bf16 matmul)
               Es = exp(sc_psum)                 (ACT, bf16)
               P  = Es * En                      (DVE 2x bf16)
  ctx^T[d,q] (+sums row) = sum_k [V|1]^T P       (bf16 matmul, PSUM accum)
  ctx normalized via reciprocal_approx_fast on an SBUF copy of sums
  out[q,o] partial = ctxT^T @ WoT (bf16)  -> DMA to DRAM (f32)
"""

import math
import os
import sys

import numpy as np

for _p in ("/opt/trn_rl_repo",):
    if _p not in sys.path and os.path.isdir(_p):
        sys.path.insert(0, _p)

os.environ.setdefault("MYCRO_LOCAL_CACHE", "1")

import ml_dtypes

import concourse.bass as bass
import concourse.tile as tile
from concourse import bacc, mybir
from concourse.bass_utils import run_bass_kernel_spmd
from concourse.masks import make_identity


def _install_ntff_hook():
    """Provide antenv.axon_hooks (NTFF profiling registry) if the image
    lacks it, mirroring trn_agent_boot's ctypes hook. No-op on failure."""
    try:
        import antenv.axon_hooks  # noqa: F401
        return
    except ImportError:
        pass
    try:
        import contextlib
        import ctypes
        import types

        so_path = "/opt/axon/libaxon_pjrt.so"
        if not os.path.exists(so_path):
            return
        lib = ctypes.CDLL(so_path)
        if not hasattr(lib, "axon_start_nrt_profile"):
            return
        lib.axon_start_nrt_profile.argtypes = [
            ctypes.POINTER(ctypes.c_int64), ctypes.c_size_t]
        lib.axon_start_nrt_profile.restype = ctypes.c_int64
        lib.axon_stop_nrt_profile.argtypes = [ctypes.c_char_p]
        lib.axon_stop_nrt_profile.restype = ctypes.c_int64

        @contextlib.contextmanager
        def _hook(output_dir, device_ids):
            import jax
            jax.devices()
            if device_ids:
                ids = (ctypes.c_int64 * len(device_ids))(*device_ids)
                rc = lib.axon_start_nrt_profile(ids, len(device_ids))
            else:
                rc = lib.axon_start_nrt_profile(None, 0)
            if rc != 0:
                raise RuntimeError(f"axon_start_nrt_profile rc={rc}")
            try:
                yield
            finally:
                n = lib.axon_stop_nrt_profile(str(output_dir).encode())
                print(f"ntff profile: {n} file(s) -> {output_dir}",
                      file=sys.stderr)

        mod = types.ModuleType("antenv.axon_hooks")
        _state = {"hook": _hook}
        mod.set_axon_ntff_profile_hook = lambda h: _state.__setitem__("hook", h)
        mod.get_axon_ntff_profile_hook = lambda: _state["hook"]
        sys.modules["antenv.axon_hooks"] = mod
        import antenv
        antenv.axon_hooks = mod
    except Exception:
        pass


_install_ntff_hook()

F32 = mybir.dt.float32
BF16 = mybir.dt.bfloat16
FP8 = mybir.dt.float8e4
ALU = mybir.AluOpType
ACT_EXP = mybir.ActivationFunctionType.Exp
ACT_COPY = mybir.ActivationFunctionType.Copy
DR = mybir.MatmulPerfMode.DoubleRow

# Problem constants (hardcoded per contract).
HIDDEN = 1024
HEADS = 16
HEAD_DIM = 64
GAMMA = 0.5
B, S = 2, 2048
N_CORES = 8
GROUPS = N_CORES // B  # head groups per batch
HPC = HEADS // GROUPS  # heads per core
LAG = 4  # kt software-pipeline lag between P and ctx matmul
FP8_SCALE = 64.0  # host scales normalized x by this before fp8 cast
VW = 66  # v2 padded row width (64 dims + ones col + pad)
SIM_DR = True  # fp8 DoubleRow for the sim matmuls


def emit_kernel(tc, aps, *, S_, C_, HPC_, QB, with_mask, debug_taps=False):
    """Emit the per-core kernel. aps: dict of dram APs."""
    nc = tc.nc
    CT = C_ // 128          # contraction chunks over hidden
    PAIRS = HPC_ // 2       # head pairs (128-channel chunks)
    NKT = S_ // 128         # key tiles
    NQB = S_ // QB          # query blocks
    PB = min(512, S_)       # projection free-block width
    NPB = S_ // PB
    OB_W = min(512, C_)     # out-projection free-block width
    NOB = C_ // OB_W
    D2 = HPC_ * HEAD_DIM
    neg_gamma_scale = -GAMMA / (FP8_SCALE * FP8_SCALE)

    xbf_d = aps["xbf"]; xh8_d = aps["xh8"]
    wq_d = aps["wq"]; wk_d = aps["wk"]; wv_d = aps["wv"]; wo_d = aps["wo"]
    bq_d = aps["bq"]; bk_d = aps["bk"]; bv_d = aps["bv"]
    out_d = aps["out"]
    m01_d = aps.get("mask01")

    from contextlib import ExitStack
    stack = ExitStack()
    consts = stack.enter_context(tc.tile_pool(name="consts", bufs=1))
    xpool = stack.enter_context(tc.tile_pool(name="xpool", bufs=1))
    projpool = stack.enter_context(tc.tile_pool(name="projpool", bufs=1))

    identity = consts.tile([128, 128], BF16)
    make_identity(nc, identity)

    wo_sb = consts.tile([128, PAIRS, C_], BF16)
    nc.sync.dma_start(out=wo_sb, in_=wo_d.rearrange("(j p) o -> p j o", p=128))

    # x^T in bf16 (projections) and fp8 (sim), both host-precast
    xbf_sb = xpool.tile([128, CT, S_], BF16)
    xh8_sb = xpool.tile([128, CT, S_], FP8)
    for c in range(CT):
        nc.sync.dma_start(out=xbf_sb[:, c, :], in_=xbf_d[c * 128:(c + 1) * 128, :])
    for c in range(CT):
        nc.sync.dma_start(out=xh8_sb[:, c, :], in_=xh8_d[c * 128:(c + 1) * 128, :])

    # projections
    qT_sb = projpool.tile([128, PAIRS, S_], BF16)
    kT_sb = projpool.tile([128, PAIRS, S_], BF16)
    v2_sb = projpool.tile([128, HPC_, NKT, VW], BF16)

    with tc.tile_pool(name="wpool", bufs=1) as wpool, \
         tc.tile_pool(name="vstage", bufs=1) as vstage, \
         tc.tile_pool(name="ph1psum", bufs=2, space="PSUM") as prj_ps, \
         tc.tile_pool(name="tppsum", bufs=4, space="PSUM") as tp_ps:
        wq_sb = wpool.tile([128, CT, D2], BF16)
        wk_sb = wpool.tile([128, CT, D2], BF16)
        wv_sb = wpool.tile([128, CT, D2], BF16)
        for w_sb, w_d in ((wq_sb, wq_d), (wk_sb, wk_d), (wv_sb, wv_d)):
            nc.sync.dma_start(out=w_sb, in_=w_d.rearrange("(t p) m -> p t m", p=128))
        bq_sb = wpool.tile([128, PAIRS, 1], F32)
        bk_sb = wpool.tile([128, PAIRS, 1], F32)
        bv_sb = wpool.tile([128, PAIRS, 1], F32)
        for b_sb, b_d in ((bq_sb, bq_d), (bk_sb, bk_d), (bv_sb, bv_d)):
            nc.sync.dma_start(
                out=b_sb, in_=b_d.rearrange("(j p) one -> p j one", p=128))

        vT_sb = vstage.tile([128, PAIRS, S_], BF16)
        for w_sb, b_sb, dest in (
            (wq_sb, bq_sb, qT_sb),
            (wk_sb, bk_sb, kT_sb),
            (wv_sb, bv_sb, vT_sb),
        ):
            for nb in range(NPB):
                pss = [prj_ps.tile([128, PB], F32, tag=f"prj{j}",
                                   name=f"prj_{dest.tensor.name}_{nb}_{j}")
                       for j in range(PAIRS)]
                for c in range(CT):
                    for j in range(PAIRS):
                        nc.tensor.matmul(
                            pss[j],
                            w_sb[:, c, j * 128:(j + 1) * 128],
                            xbf_sb[:, c, nb * PB:(nb + 1) * PB],
                            start=(c == 0),
                            stop=(c == CT - 1),
                        )
                for j in range(PAIRS):
                    nc.vector.tensor_scalar_add(
                        dest[:, j, nb * PB:(nb + 1) * PB], pss[j], b_sb[:, j, :]
                    )

        if debug_taps:
            nc.sync.dma_start(out=aps["dbg_qT"], in_=qT_sb)
            nc.sync.dma_start(out=aps["dbg_kT"], in_=kT_sb)

        # V: PE-transpose vT (bf16) -> [keys, d] layout, 2 heads per tile
        for j in range(PAIRS):
            for t in range(NKT):
                tp = tp_ps.tile([128, 128], BF16, tag="tp")
                nc.tensor.transpose(tp, vT_sb[:, j, t * 128:(t + 1) * 128], identity)
                nc.vector.tensor_copy(
                    v2_sb[:, 2 * j:2 * j + 2, t, 0:HEAD_DIM],
                    tp.rearrange("p (h d) -> p h d", h=2),
                )
        nc.vector.memset(v2_sb[:, :, :, HEAD_DIM:HEAD_DIM + 1], 1.0)
        if debug_taps:
            nc.sync.dma_start(out=aps["dbg_v2"],
                              in_=v2_sb[:, :, :, 0:HEAD_DIM + 1])

    # --- main loop (phase 2) ---
    ctxT2_sb = projpool.tile([128, PAIRS, S_], BF16)
    ptpool = stack.enter_context(tc.tile_pool(name="ptpool", bufs=22))
    espool = stack.enter_context(tc.tile_pool(name="espool", bufs=4))
    enpool = stack.enter_context(tc.tile_pool(name="enpool", bufs=3))
    smallpool = stack.enter_context(tc.tile_pool(name="smallpool", bufs=2))
    mpool = (stack.enter_context(tc.tile_pool(name="mpool", bufs=2))
             if m01_d is not None else None)

    with tc.tile_pool(name="simpsum", bufs=1, space="PSUM") as simp, \
         tc.tile_pool(name="scpsum", bufs=3, space="PSUM") as scp, \
         tc.tile_pool(name="ctxpsum", bufs=1, space="PSUM") as ctxp:

        def emit_ctx(ctx_ps, kt, pts):
            for h in range(HPC_):
                nc.tensor.matmul(
                    ctx_ps[h],
                    v2_sb[:, h, kt, 0:HEAD_DIM + 1],
                    pts[h],
                    start=(kt == 0),
                    stop=(kt == NKT - 1),
                    skip_group_check=True,
                )

        def emit_division_head(qb, ctx_ps, h):
            j, hi = divmod(h, 2)
            s_sb = smallpool.tile([1, QB], F32, tag=f"ssb{h % 2}",
                                  name=f"ssb_{qb}_{h}")
            nc.vector.tensor_copy(s_sb, ctx_ps[h][HEAD_DIM:HEAD_DIM + 1, :])
            r0 = smallpool.tile([1, QB], F32, tag=f"r0{h % 2}",
                                name=f"r0_{qb}_{h}")
            nc.vector.reciprocal_approx_fast(r0, s_sb)
            if debug_taps and qb == 0 and h == 0:
                nc.sync.dma_start(out=aps["dbg_r0"], in_=r0)
            rb = smallpool.tile([HEAD_DIM, QB], F32, tag="rb")
            nc.gpsimd.partition_broadcast(rb, r0, channels=HEAD_DIM)
            nc.vector.tensor_mul(
                ctxT2_sb[hi * 64:hi * 64 + 64, j, qb * QB:(qb + 1) * QB],
                ctx_ps[h][0:HEAD_DIM, :],
                rb,
            )

        def emit_division(qb, ctx_ps):
            for h in range(HPC_):
                emit_division_head(qb, ctx_ps, h)

        prev_div = None
        for qb in range(NQB):
            ctx_ps = [ctxp.tile([HEAD_DIM + 1, QB], F32, tag=f"ctx{h}",
                                name=f"ctx_{qb}_{h}")
                      for h in range(HPC_)]
            pending = []
            for kt in range(NKT):
                if prev_div is not None and kt < HPC_:
                    emit_division_head(prev_div[0], prev_div[1], kt)
                    if kt == HPC_ - 1:
                        prev_div = None
                # sim via fp8 DoubleRow: 2 hidden-chunks per pass
                sp = simp.tile([128, QB], F32, tag="sim")
                if SIM_DR:
                    for c2 in range(CT // 2):
                        nc.tensor.matmul(
                            sp,
                            xh8_sb[:, 2 * c2:2 * c2 + 2, kt * 128:(kt + 1) * 128],
                            xh8_sb[:, 2 * c2:2 * c2 + 2, qb * QB:(qb + 1) * QB],
                            start=(c2 == 0),
                            stop=(c2 == CT // 2 - 1),
                            perf_mode=DR,
                        )
                else:
                    for c in range(CT):
                        nc.tensor.matmul(
                            sp,
                            xh8_sb[:, c, kt * 128:(kt + 1) * 128],
                            xh8_sb[:, c, qb * QB:(qb + 1) * QB],
                            start=(c == 0),
                            stop=(c == CT - 1),
                        )
                en = enpool.tile([128, QB], BF16, tag="en")
                nc.scalar.activation(out=en, in_=sp, func=ACT_EXP,
                                     scale=neg_gamma_scale)
                if debug_taps and qb == 0 and kt == 0:
                    nc.sync.dma_start(out=aps["dbg_en0"], in_=en)
                if m01_d is not None:
                    m_sb = mpool.tile([128, QB], BF16, tag="msk")
                    nc.sync.dma_start(
                        out=m_sb,
                        in_=m01_d[kt * 128:(kt + 1) * 128, qb * QB:(qb + 1) * QB],
                    )
                    nc.vector.tensor_mul(en, en, m_sb)
                pts = []
                for h in range(HPC_):
                    j, hi = divmod(h, 2)
                    pr = slice(hi * 64, hi * 64 + 64)
                    sc_t = scp.tile([128, QB], F32, tag="sc")
                    nc.tensor.matmul(
                        sc_t,
                        kT_sb[pr, j, kt * 128:(kt + 1) * 128],
                        qT_sb[pr, j, qb * QB:(qb + 1) * QB],
                        start=True,
                        stop=True,
                    )
                    es = espool.tile([128, QB], BF16, tag="es")
                    nc.scalar.activation(out=es, in_=sc_t, func=ACT_EXP)
                    pt = ptpool.tile([128, QB], BF16, tag="pt")
                    nc.vector.tensor_mul(pt, es, en)
                    if debug_taps and qb == 0 and kt == 0 and h == 0:
                        nc.sync.dma_start(out=aps["dbg_pt0"], in_=pt)
                    pts.append(pt)
                pending.append((kt, pts))
                if len(pending) > LAG:
                    k0, p0 = pending.pop(0)
                    emit_ctx(ctx_ps, k0, p0)
            for k0, p0 in pending:
                emit_ctx(ctx_ps, k0, p0)
            if prev_div is not None:
                for h in range(min(NKT, HPC_), HPC_):
                    emit_division_head(prev_div[0], prev_div[1], h)
                prev_div = None
            prev_div = (qb, ctx_ps)
        emit_division(*prev_div)

    if debug_taps:
        nc.sync.dma_start(out=aps["dbg_ctxT2"], in_=ctxT2_sb)

    # --- out-projection (phase 3) ---
    with tc.tile_pool(name="outpsum", bufs=4, space="PSUM") as outp, \
         tc.tile_pool(name="outstg", bufs=4) as outstg:
        for qt in range(S_ // 128):
            for ob in range(NOB):
                op = outp.tile([128, OB_W], F32, tag="op")
                for j in range(PAIRS):
                    nc.tensor.matmul(
                        op,
                        ctxT2_sb[:, j, qt * 128:(qt + 1) * 128],
                        wo_sb[:, j, ob * OB_W:(ob + 1) * OB_W],
                        start=(j == 0),
                        stop=(j == PAIRS - 1),
                    )
                ostg = outstg.tile([128, OB_W], F32, tag="ostg")
                nc.vector.tensor_copy(ostg, op)
                nc.sync.dma_start(
                    out=out_d[qt * 128:(qt + 1) * 128, ob * OB_W:(ob + 1) * OB_W],
                    in_=ostg,
                )

    stack.close()


def build_nc(*, S_=S, C_=HIDDEN, HPC_=HPC, QB=512, with_mask=False,
             enable_asserts=False, debug_taps=False):
    nc = bacc.Bacc(
        "TRN2", target_bir_lowering=False, debug=False,
        enable_asserts=enable_asserts,
    )
    D2 = HPC_ * HEAD_DIM
    PAIRS = HPC_ // 2
    NKT = S_ // 128
    aps = {}
    aps["xbf"] = nc.dram_tensor("xbf", [C_, S_], BF16, kind="ExternalInput").ap()
    aps["xh8"] = nc.dram_tensor("xh8", [C_, S_], FP8, kind="ExternalInput").ap()
    for n in ("wq", "wk", "wv"):
        aps[n] = nc.dram_tensor(n, [C_, D2], BF16, kind="ExternalInput").ap()
    aps["wo"] = nc.dram_tensor("wo", [D2, C_], BF16, kind="ExternalInput").ap()
    for n in ("bq", "bk", "bv"):
        aps[n] = nc.dram_tensor(n, [D2, 1], F32, kind="ExternalInput").ap()
    if with_mask:
        aps["mask01"] = nc.dram_tensor(
            "mask01", [S_, S_], BF16, kind="ExternalInput").ap()
    aps["out"] = nc.dram_tensor("out", [S_, C_], F32, kind="ExternalOutput").ap()
    if debug_taps:
        aps["dbg_qT"] = nc.dram_tensor(
            "dbg_qT", [128, PAIRS, S_], BF16, kind="ExternalOutput").ap()
        aps["dbg_kT"] = nc.dram_tensor(
            "dbg_kT", [128, PAIRS, S_], BF16, kind="ExternalOutput").ap()
        aps["dbg_v2"] = nc.dram_tensor(
            "dbg_v2", [128, HPC_, NKT, HEAD_DIM + 1], BF16,
            kind="ExternalOutput").ap()
        aps["dbg_ctxT2"] = nc.dram_tensor(
            "dbg_ctxT2", [128, PAIRS, S_], BF16, kind="ExternalOutput").ap()
        aps["dbg_en0"] = nc.dram_tensor(
            "dbg_en0", [128, QB], BF16, kind="ExternalOutput").ap()
        aps["dbg_pt0"] = nc.dram_tensor(
            "dbg_pt0", [128, QB], BF16, kind="ExternalOutput").ap()
        aps["dbg_r0"] = nc.dram_tensor(
            "dbg_r0", [1, QB], F32, kind="ExternalOutput").ap()

    with tile.TileContext(nc) as tc:
        emit_kernel(tc, aps, S_=S_, C_=C_, HPC_=HPC_, QB=QB,
                    with_mask=with_mask, debug_taps=debug_taps)
    nc.compile()
    return nc


def host_prepare(x, attn_mask, Wq, bq, Wk, bk, Wv, bv, Wo, bo, *,
                 S_=S, C_=HIDDEN, HPC_=HPC, n_cores=N_CORES):
    """Build the per-core input maps. Returns (in_maps, with_mask)."""
    bf = ml_dtypes.bfloat16
    f8 = ml_dtypes.float8_e4m3fn
    x = np.asarray(x, np.float32)
    B_ = x.shape[0]
    groups = n_cores // B_
    Wq = np.asarray(Wq, np.float32); Wk = np.asarray(Wk, np.float32)
    Wv = np.asarray(Wv, np.float32); Wo = np.asarray(Wo, np.float32)
    bq = np.asarray(bq, np.float32); bk = np.asarray(bk, np.float32)
    bv = np.asarray(bv, np.float32)

    inv_sqrt_d = 1.0 / math.sqrt(HEAD_DIM)
    WqT = np.ascontiguousarray((Wq * inv_sqrt_d).T).astype(bf)  # [C, C]
    WkT = np.ascontiguousarray(Wk.T).astype(bf)
    WvT = np.ascontiguousarray(Wv.T).astype(bf)
    WoT = np.ascontiguousarray(Wo.T).astype(bf)                 # [C(c), C(o)]
    bq = bq * inv_sqrt_d

    mask = np.asarray(attn_mask)
    with_mask = bool(mask.any())
    mask01 = None
    if with_mask:
        # reference: where(mask, -inf) -> multiplicative 0/1 on P
        mask01 = np.where(mask, 0.0, 1.0).astype(bf)
        mask01 = np.ascontiguousarray(mask01.T)  # [k, q]

    in_maps = []
    for core in range(n_cores):
        b, g = divmod(core, groups)
        xb = x[b]                                   # [S, C]
        xT = np.ascontiguousarray(xb.T)             # [C, S]
        norms = np.linalg.norm(xb, axis=1)          # [S]
        scale = (FP8_SCALE / np.maximum(norms, 1e-12)).astype(np.float32)
        xh8 = (xT * scale[None, :]).astype(f8)
        ch = slice(g * HPC_ * HEAD_DIM, (g + 1) * HPC_ * HEAD_DIM)
        m = {
            "xbf": xT.astype(bf),
            "xh8": xh8,
            "wq": np.ascontiguousarray(WqT[:, ch]),
            "wk": np.ascontiguousarray(WkT[:, ch]),
            "wv": np.ascontiguousarray(WvT[:, ch]),
            "wo": np.ascontiguousarray(WoT[ch, :]),
            "bq": np.ascontiguousarray(bq[ch]).reshape(-1, 1),
            "bk": np.ascontiguousarray(bk[ch]).reshape(-1, 1),
            "bv": np.ascontiguousarray(bv[ch]).reshape(-1, 1),
        }
        if with_mask:
            m["mask01"] = mask01
        in_maps.append(m)
    return in_maps, with_mask


_NC_CACHE = {}


def _get_nc(with_mask):
    key = with_mask
    if key not in _NC_CACHE:
        _NC_CACHE[key] = build_nc(with_mask=with_mask)
    return _NC_CACHE[key]


LAST_RESULTS = None


def kernel(**inputs):
    global LAST_RESULTS
    in_maps, with_mask = host_prepare(
        inputs["x"], inputs["attn_mask"],
        inputs["Wq"], inputs["bq"], inputs["Wk"], inputs["bk"],
        inputs["Wv"], inputs["bv"], inputs["Wo"], inputs["bo"],
    )
    nc = _get_nc(with_mask)
    res = run_bass_kernel_spmd(nc, in_maps, core_ids=list(range(N_CORES)))
    LAST_RESULTS = res
    bo = np.asarray(inputs["bo"], np.float32)
    out = np.zeros((B, S, HIDDEN), np.float32)
    groups = N_CORES // B
    for core in range(N_CORES):
        b = core // groups
        out[b] += res.results[core]["out"]
    out += bo[None, None, :]
    return out


# revision 23
# speedup vs baseline: 1.6507x; 1.0451x over previous
"""DiversityAttention on 8 TRN2 NeuronCores (Bass/Tile), bf16/fp8 edition.

Sharding: data-parallel over batch (B=2) x tensor-parallel over heads
(16 heads -> 4 groups of 4). core = (b, g), b = core // 4, g = core % 4.
Each core computes full attention for its 4 heads over its batch and a
partial out-projection [S, HIDDEN]; the host sums the 4 partials per
batch and adds bo.

Device-side formulation (keys-on-partitions / "S^T" orientation):
  qT = (Wq/sqrt(dh) @ x^T + bq')  [64h, S]  bf16
  kT = (Wk @ x^T + bk)            [64h, S]  bf16
  vT = (Wv @ x^T + bv) then PE-transposed to V [S, 64h] bf16 (+ ones col)
  xh8 = fp8(64 * x^T / max(||x||, eps))  (host-precomputed)
  per (qb, kt):
     sim_psum[k,q] = xh8^T xh8   (fp8 DoubleRow matmuls, 2 chunks/pass)
     En = exp(-gamma/4096 * sim_psum)            (ACT, bf16)
     per head: sc_psum[k,q] = kT^T qT            (bf16 matmul)
               Es = exp(sc_psum)                 (ACT, bf16)
               P  = Es * En                      (DVE 2x bf16)
  ctx^T[d,q] (+sums row) = sum_k [V|1]^T P       (bf16 matmul, PSUM accum)
  ctx normalized via reciprocal_approx_fast on an SBUF copy of sums
  out[q,o] partial = ctxT^T @ WoT (bf16)  -> DMA to DRAM (f32)
"""

import math
import os
import sys

import numpy as np

for _p in ("/opt/trn_rl_repo",):
    if _p not in sys.path and os.path.isdir(_p):
        sys.path.insert(0, _p)

os.environ.setdefault("MYCRO_LOCAL_CACHE", "1")

import ml_dtypes

import concourse.bass as bass
import concourse.tile as tile
from concourse import bacc, mybir
from concourse.bass_utils import run_bass_kernel_spmd
from concourse.masks import make_identity


def _install_ntff_hook():
    """Provide antenv.axon_hooks (NTFF profiling registry) if the image
    lacks it, mirroring trn_agent_boot's ctypes hook. No-op on failure."""
    try:
        import antenv.axon_hooks  # noqa: F401
        return
    except ImportError:
        pass
    try:
        import contextlib
        import ctypes
        import types

        so_path = "/opt/axon/libaxon_pjrt.so"
        if not os.path.exists(so_path):
            return
        lib = ctypes.CDLL(so_path)
        if not hasattr(lib, "axon_start_nrt_profile"):
            return
        lib.axon_start_nrt_profile.argtypes = [
            ctypes.POINTER(ctypes.c_int64), ctypes.c_size_t]
        lib.axon_start_nrt_profile.restype = ctypes.c_int64
        lib.axon_stop_nrt_profile.argtypes = [ctypes.c_char_p]
        lib.axon_stop_nrt_profile.restype = ctypes.c_int64

        @contextlib.contextmanager
        def _hook(output_dir, device_ids):
            import jax
            jax.devices()
            if device_ids:
                ids = (ctypes.c_int64 * len(device_ids))(*device_ids)
                rc = lib.axon_start_nrt_profile(ids, len(device_ids))
            else:
                rc = lib.axon_start_nrt_profile(None, 0)
            if rc != 0:
                raise RuntimeError(f"axon_start_nrt_profile rc={rc}")
            try:
                yield
            finally:
                n = lib.axon_stop_nrt_profile(str(output_dir).encode())
                print(f"ntff profile: {n} file(s) -> {output_dir}",
                      file=sys.stderr)

        mod = types.ModuleType("antenv.axon_hooks")
        _state = {"hook": _hook}
        mod.set_axon_ntff_profile_hook = lambda h: _state.__setitem__("hook", h)
        mod.get_axon_ntff_profile_hook = lambda: _state["hook"]
        sys.modules["antenv.axon_hooks"] = mod
        import antenv
        antenv.axon_hooks = mod
    except Exception:
        pass


_install_ntff_hook()

F32 = mybir.dt.float32
BF16 = mybir.dt.bfloat16
FP8 = mybir.dt.float8e4
ALU = mybir.AluOpType
ACT_EXP = mybir.ActivationFunctionType.Exp
ACT_COPY = mybir.ActivationFunctionType.Copy
DR = mybir.MatmulPerfMode.DoubleRow

# Problem constants (hardcoded per contract).
HIDDEN = 1024
HEADS = 16
HEAD_DIM = 64
GAMMA = 0.5
B, S = 2, 2048
N_CORES = 8
GROUPS = N_CORES // B  # head groups per batch
HPC = HEADS // GROUPS  # heads per core
LAG = 4  # kt software-pipeline lag between P and ctx matmul
FP8_SCALE = 64.0  # host scales normalized x by this before fp8 cast
VW = 66  # v2 padded row width (64 dims + ones col + pad)
SIM_DR = True  # fp8 DoubleRow for the sim matmuls


def emit_kernel(tc, aps, *, S_, C_, HPC_, QB, with_mask, debug_taps=False):
    """Emit the per-core kernel. aps: dict of dram APs."""
    nc = tc.nc
    CT = C_ // 128          # contraction chunks over hidden
    PAIRS = HPC_ // 2       # head pairs (128-channel chunks)
    NKT = S_ // 128         # key tiles
    NQB = S_ // QB          # query blocks
    PB = min(512, S_)       # projection free-block width
    NPB = S_ // PB
    OB_W = min(512, C_)     # out-projection free-block width
    NOB = C_ // OB_W
    D2 = HPC_ * HEAD_DIM
    neg_gamma_scale = -GAMMA / (FP8_SCALE * FP8_SCALE)

    xbf_d = aps["xbf"]; xh8_d = aps["xh8"]
    wq_d = aps["wq"]; wk_d = aps["wk"]; wv_d = aps["wv"]; wo_d = aps["wo"]
    bq_d = aps["bq"]; bk_d = aps["bk"]; bv_d = aps["bv"]
    out_d = aps["out"]
    m01_d = aps.get("mask01")

    from contextlib import ExitStack
    stack = ExitStack()
    consts = stack.enter_context(tc.tile_pool(name="consts", bufs=1))
    xpool = stack.enter_context(tc.tile_pool(name="xpool", bufs=1))
    projpool = stack.enter_context(tc.tile_pool(name="projpool", bufs=1))

    identity = consts.tile([128, 128], BF16)
    make_identity(nc, identity)

    wo_sb = consts.tile([128, PAIRS, C_], BF16)

    # x^T in bf16 (projections) and fp8 (sim), both host-precast
    xbf_sb = xpool.tile([128, CT, S_], BF16)
    xh8_sb = xpool.tile([128, CT, S_], FP8)

    # projections
    qT_sb = projpool.tile([128, PAIRS, S_], BF16)
    kT_sb = projpool.tile([128, PAIRS, S_], BF16)
    v2_sb = projpool.tile([128, HPC_, NKT, VW], BF16)

    with tc.tile_pool(name="wpool", bufs=1) as wpool, \
         tc.tile_pool(name="vstage", bufs=1) as vstage, \
         tc.tile_pool(name="ph1psum", bufs=2, space="PSUM") as prj_ps, \
         tc.tile_pool(name="tppsum", bufs=4, space="PSUM") as tp_ps:
        wq_sb = wpool.tile([128, CT, D2], BF16)
        wk_sb = wpool.tile([128, CT, D2], BF16)
        wv_sb = wpool.tile([128, CT, D2], BF16)
        bq_sb = wpool.tile([128, PAIRS, 1], F32)
        bk_sb = wpool.tile([128, PAIRS, 1], F32)
        bv_sb = wpool.tile([128, PAIRS, 1], F32)
        # DMA order = arrival order: wq first so projections start early,
        # xh8/wo last (needed only by the main loop / out-projection).
        nc.sync.dma_start(out=wq_sb, in_=wq_d.rearrange("(t p) m -> p t m", p=128))
        nc.sync.dma_start(out=bq_sb, in_=bq_d.rearrange("(j p) one -> p j one", p=128))
        for c in range(CT):
            nc.sync.dma_start(out=xbf_sb[:, c, :],
                              in_=xbf_d[c * 128:(c + 1) * 128, :])
        for w_sb, b_sb, w_d, b_d in ((wk_sb, bk_sb, wk_d, bk_d),
                                     (wv_sb, bv_sb, wv_d, bv_d)):
            nc.sync.dma_start(out=w_sb, in_=w_d.rearrange("(t p) m -> p t m", p=128))
            nc.sync.dma_start(out=b_sb,
                              in_=b_d.rearrange("(j p) one -> p j one", p=128))
        for c in range(CT):
            nc.sync.dma_start(out=xh8_sb[:, c, :],
                              in_=xh8_d[c * 128:(c + 1) * 128, :])
        nc.sync.dma_start(out=wo_sb, in_=wo_d.rearrange("(j p) o -> p j o", p=128))

        vT_sb = vstage.tile([128, PAIRS, S_], BF16)
        for w_sb, b_sb, dest in (
            (wq_sb, bq_sb, qT_sb),
            (wk_sb, bk_sb, kT_sb),
            (wv_sb, bv_sb, vT_sb),
        ):
            for nb in range(NPB):
                pss = [prj_ps.tile([128, PB], F32, tag=f"prj{j}",
                                   name=f"prj_{dest.tensor.name}_{nb}_{j}")
                       for j in range(PAIRS)]
                for c in range(CT):
                    for j in range(PAIRS):
                        nc.tensor.matmul(
                            pss[j],
                            w_sb[:, c, j * 128:(j + 1) * 128],
                            xbf_sb[:, c, nb * PB:(nb + 1) * PB],
                            start=(c == 0),
                            stop=(c == CT - 1),
                        )
                for j in range(PAIRS):
                    nc.vector.tensor_scalar_add(
                        dest[:, j, nb * PB:(nb + 1) * PB], pss[j], b_sb[:, j, :]
                    )

        if debug_taps:
            nc.sync.dma_start(out=aps["dbg_qT"], in_=qT_sb)
            nc.sync.dma_start(out=aps["dbg_kT"], in_=kT_sb)

        # V: PE-transpose vT (bf16) -> [keys, d] layout, 2 heads per tile
        for j in range(PAIRS):
            for t in range(NKT):
                tp = tp_ps.tile([128, 128], BF16, tag="tp")
                nc.tensor.transpose(tp, vT_sb[:, j, t * 128:(t + 1) * 128], identity)
                nc.vector.tensor_copy(
                    v2_sb[:, 2 * j:2 * j + 2, t, 0:HEAD_DIM],
                    tp.rearrange("p (h d) -> p h d", h=2),
                )
        nc.vector.memset(v2_sb[:, :, :, HEAD_DIM:HEAD_DIM + 1], 1.0)
        if debug_taps:
            nc.sync.dma_start(out=aps["dbg_v2"],
                              in_=v2_sb[:, :, :, 0:HEAD_DIM + 1])

    # --- main loop (phase 2) ---
    ctxT2_sb = projpool.tile([128, PAIRS, S_], BF16)
    ptpool = stack.enter_context(tc.tile_pool(name="ptpool", bufs=22))
    espool = stack.enter_context(tc.tile_pool(name="espool", bufs=4))
    enpool = stack.enter_context(tc.tile_pool(name="enpool", bufs=3))
    smallpool = stack.enter_context(tc.tile_pool(name="smallpool", bufs=2))
    mpool = (stack.enter_context(tc.tile_pool(name="mpool", bufs=2))
             if m01_d is not None else None)

    with tc.tile_pool(name="simpsum", bufs=1, space="PSUM") as simp, \
         tc.tile_pool(name="scpsum", bufs=3, space="PSUM") as scp, \
         tc.tile_pool(name="ctxpsum", bufs=1, space="PSUM") as ctxp:

        def emit_ctx(ctx_ps, kt, pts):
            for h in range(HPC_):
                nc.tensor.matmul(
                    ctx_ps[h],
                    v2_sb[:, h, kt, 0:HEAD_DIM + 1],
                    pts[h],
                    start=(kt == 0),
                    stop=(kt == NKT - 1),
                    skip_group_check=True,
                )

        def emit_division_head(qb, ctx_ps, h):
            j, hi = divmod(h, 2)
            s_sb = smallpool.tile([1, QB], F32, tag=f"ssb{h % 2}",
                                  name=f"ssb_{qb}_{h}")
            nc.vector.tensor_copy(s_sb, ctx_ps[h][HEAD_DIM:HEAD_DIM + 1, :])
            r0 = smallpool.tile([1, QB], F32, tag=f"r0{h % 2}",
                                name=f"r0_{qb}_{h}")
            nc.vector.reciprocal_approx_fast(r0, s_sb)
            if debug_taps and qb == 0 and h == 0:
                nc.sync.dma_start(out=aps["dbg_r0"], in_=r0)
            rb = smallpool.tile([HEAD_DIM, QB], F32, tag="rb")
            nc.gpsimd.partition_broadcast(rb, r0, channels=HEAD_DIM)
            nc.vector.tensor_mul(
                ctxT2_sb[hi * 64:hi * 64 + 64, j, qb * QB:(qb + 1) * QB],
                ctx_ps[h][0:HEAD_DIM, :],
                rb,
            )

        def emit_division(qb, ctx_ps):
            for h in range(HPC_):
                emit_division_head(qb, ctx_ps, h)

        prev_div = None
        for qb in range(NQB):
            ctx_ps = [ctxp.tile([HEAD_DIM + 1, QB], F32, tag=f"ctx{h}",
                                name=f"ctx_{qb}_{h}")
                      for h in range(HPC_)]
            pending = []
            for kt in range(NKT):
                if prev_div is not None and kt < HPC_:
                    emit_division_head(prev_div[0], prev_div[1], kt)
                    if kt == HPC_ - 1:
                        prev_div = None
                # sim via fp8 DoubleRow: 2 hidden-chunks per pass
                sp = simp.tile([128, QB], F32, tag="sim")
                if SIM_DR:
                    for c2 in range(CT // 2):
                        nc.tensor.matmul(
                            sp,
                            xh8_sb[:, 2 * c2:2 * c2 + 2, kt * 128:(kt + 1) * 128],
                            xh8_sb[:, 2 * c2:2 * c2 + 2, qb * QB:(qb + 1) * QB],
                            start=(c2 == 0),
                            stop=(c2 == CT // 2 - 1),
                            perf_mode=DR,
                        )
                else:
                    for c in range(CT):
                        nc.tensor.matmul(
                            sp,
                            xh8_sb[:, c, kt * 128:(kt + 1) * 128],
                            xh8_sb[:, c, qb * QB:(qb + 1) * QB],
                            start=(c == 0),
                            stop=(c == CT - 1),
                        )
                en = enpool.tile([128, QB], BF16, tag="en")
                nc.scalar.activation(out=en, in_=sp, func=ACT_EXP,
                                     scale=neg_gamma_scale)
                if debug_taps and qb == 0 and kt == 0:
                    nc.sync.dma_start(out=aps["dbg_en0"], in_=en)
                if m01_d is not None:
                    m_sb = mpool.tile([128, QB], BF16, tag="msk")
                    nc.sync.dma_start(
                        out=m_sb,
                        in_=m01_d[kt * 128:(kt + 1) * 128, qb * QB:(qb + 1) * QB],
                    )
                    nc.vector.tensor_mul(en, en, m_sb)
                pts = []
                for h in range(HPC_):
                    j, hi = divmod(h, 2)
                    pr = slice(hi * 64, hi * 64 + 64)
                    sc_t = scp.tile([128, QB], F32, tag="sc")
                    nc.tensor.matmul(
                        sc_t,
                        kT_sb[pr, j, kt * 128:(kt + 1) * 128],
                        qT_sb[pr, j, qb * QB:(qb + 1) * QB],
                        start=True,
                        stop=True,
                    )
                    es = espool.tile([128, QB], BF16, tag="es")
                    nc.scalar.activation(out=es, in_=sc_t, func=ACT_EXP)
                    pt = ptpool.tile([128, QB], BF16, tag="pt")
                    nc.vector.tensor_mul(pt, es, en)
                    if debug_taps and qb == 0 and kt == 0 and h == 0:
                        nc.sync.dma_start(out=aps["dbg_pt0"], in_=pt)
                    pts.append(pt)
                pending.append((kt, pts))
                if len(pending) > LAG:
                    k0, p0 = pending.pop(0)
                    emit_ctx(ctx_ps, k0, p0)
            if qb < NQB - 1:
                for k0, p0 in pending:
                    emit_ctx(ctx_ps, k0, p0)
                prev_div = (qb, ctx_ps)
            else:
                # final qb: drain head-by-head so divisions start ASAP and
                # the out-projection isn't gated on one long division tail
                for h in range(HPC_):
                    for k0, p0 in pending:
                        nc.tensor.matmul(
                            ctx_ps[h],
                            v2_sb[:, h, k0, 0:HEAD_DIM + 1],
                            p0[h],
                            start=(k0 == 0),
                            stop=(k0 == NKT - 1),
                            skip_group_check=True,
                        )
                    emit_division_head(qb, ctx_ps, h)
                prev_div = None

    if debug_taps:
        nc.sync.dma_start(out=aps["dbg_ctxT2"], in_=ctxT2_sb)

    # --- out-projection (phase 3) ---
    with tc.tile_pool(name="outpsum", bufs=4, space="PSUM") as outp, \
         tc.tile_pool(name="outstg", bufs=4) as outstg:
        for qt in range(S_ // 128):
            for ob in range(NOB):
                op = outp.tile([128, OB_W], F32, tag="op")
                for j in range(PAIRS):
                    nc.tensor.matmul(
                        op,
                        ctxT2_sb[:, j, qt * 128:(qt + 1) * 128],
                        wo_sb[:, j, ob * OB_W:(ob + 1) * OB_W],
                        start=(j == 0),
                        stop=(j == PAIRS - 1),
                    )
                ostg = outstg.tile([128, OB_W], F32, tag="ostg")
                nc.vector.tensor_copy(ostg, op)
                nc.sync.dma_start(
                    out=out_d[qt * 128:(qt + 1) * 128, ob * OB_W:(ob + 1) * OB_W],
                    in_=ostg,
                )

    stack.close()


def build_nc(*, S_=S, C_=HIDDEN, HPC_=HPC, QB=512, with_mask=False,
             enable_asserts=False, debug_taps=False):
    nc = bacc.Bacc(
        "TRN2", target_bir_lowering=False, debug=False,
        enable_asserts=enable_asserts,
    )
    D2 = HPC_ * HEAD_DIM
    PAIRS = HPC_ // 2
    NKT = S_ // 128
    aps = {}
    aps["xbf"] = nc.dram_tensor("xbf", [C_, S_], BF16, kind="ExternalInput").ap()
    aps["xh8"] = nc.dram_tensor("xh8", [C_, S_], FP8, kind="ExternalInput").ap()
    for n in ("wq", "wk", "wv"):
        aps[n] = nc.dram_tensor(n, [C_, D2], BF16, kind="ExternalInput").ap()
    aps["wo"] = nc.dram_tensor("wo", [D2, C_], BF16, kind="ExternalInput").ap()
    for n in ("bq", "bk", "bv"):
        aps[n] = nc.dram_tensor(n, [D2, 1], F32, kind="ExternalInput").ap()
    if with_mask:
        aps["mask01"] = nc.dram_tensor(
            "mask01", [S_, S_], BF16, kind="ExternalInput").ap()
    aps["out"] = nc.dram_tensor("out", [S_, C_], F32, kind="ExternalOutput").ap()
    if debug_taps:
        aps["dbg_qT"] = nc.dram_tensor(
            "dbg_qT", [128, PAIRS, S_], BF16, kind="ExternalOutput").ap()
        aps["dbg_kT"] = nc.dram_tensor(
            "dbg_kT", [128, PAIRS, S_], BF16, kind="ExternalOutput").ap()
        aps["dbg_v2"] = nc.dram_tensor(
            "dbg_v2", [128, HPC_, NKT, HEAD_DIM + 1], BF16,
            kind="ExternalOutput").ap()
        aps["dbg_ctxT2"] = nc.dram_tensor(
            "dbg_ctxT2", [128, PAIRS, S_], BF16, kind="ExternalOutput").ap()
        aps["dbg_en0"] = nc.dram_tensor(
            "dbg_en0", [128, QB], BF16, kind="ExternalOutput").ap()
        aps["dbg_pt0"] = nc.dram_tensor(
            "dbg_pt0", [128, QB], BF16, kind="ExternalOutput").ap()
        aps["dbg_r0"] = nc.dram_tensor(
            "dbg_r0", [1, QB], F32, kind="ExternalOutput").ap()

    with tile.TileContext(nc) as tc:
        emit_kernel(tc, aps, S_=S_, C_=C_, HPC_=HPC_, QB=QB,
                    with_mask=with_mask, debug_taps=debug_taps)
    nc.compile()
    return nc


def host_prepare(x, attn_mask, Wq, bq, Wk, bk, Wv, bv, Wo, bo, *,
                 S_=S, C_=HIDDEN, HPC_=HPC, n_cores=N_CORES):
    """Build the per-core input maps. Returns (in_maps, with_mask)."""
    bf = ml_dtypes.bfloat16
    f8 = ml_dtypes.float8_e4m3fn
    x = np.asarray(x, np.float32)
    B_ = x.shape[0]
    groups = n_cores // B_
    Wq = np.asarray(Wq, np.float32); Wk = np.asarray(Wk, np.float32)
    Wv = np.asarray(Wv, np.float32); Wo = np.asarray(Wo, np.float32)
    bq = np.asarray(bq, np.float32); bk = np.asarray(bk, np.float32)
    bv = np.asarray(bv, np.float32)

    inv_sqrt_d = 1.0 / math.sqrt(HEAD_DIM)
    WqT = np.ascontiguousarray((Wq * inv_sqrt_d).T).astype(bf)  # [C, C]
    WkT = np.ascontiguousarray(Wk.T).astype(bf)
    WvT = np.ascontiguousarray(Wv.T).astype(bf)
    WoT = np.ascontiguousarray(Wo.T).astype(bf)                 # [C(c), C(o)]
    bq = bq * inv_sqrt_d

    mask = np.asarray(attn_mask)
    with_mask = bool(mask.any())
    mask01 = None
    if with_mask:
        # reference: where(mask, -inf) -> multiplicative 0/1 on P
        mask01 = np.where(mask, 0.0, 1.0).astype(bf)
        mask01 = np.ascontiguousarray(mask01.T)  # [k, q]

    in_maps = []
    for core in range(n_cores):
        b, g = divmod(core, groups)
        xb = x[b]                                   # [S, C]
        xT = np.ascontiguousarray(xb.T)             # [C, S]
        norms = np.linalg.norm(xb, axis=1)          # [S]
        scale = (FP8_SCALE / np.maximum(norms, 1e-12)).astype(np.float32)
        xh8 = (xT * scale[None, :]).astype(f8)
        ch = slice(g * HPC_ * HEAD_DIM, (g + 1) * HPC_ * HEAD_DIM)
        m = {
            "xbf": xT.astype(bf),
            "xh8": xh8,
            "wq": np.ascontiguousarray(WqT[:, ch]),
            "wk": np.ascontiguousarray(WkT[:, ch]),
            "wv": np.ascontiguousarray(WvT[:, ch]),
            "wo": np.ascontiguousarray(WoT[ch, :]),
            "bq": np.ascontiguousarray(bq[ch]).reshape(-1, 1),
            "bk": np.ascontiguousarray(bk[ch]).reshape(-1, 1),
            "bv": np.ascontiguousarray(bv[ch]).reshape(-1, 1),
        }
        if with_mask:
            m["mask01"] = mask01
        in_maps.append(m)
    return in_maps, with_mask


_NC_CACHE = {}


def _get_nc(with_mask):
    key = with_mask
    if key not in _NC_CACHE:
        _NC_CACHE[key] = build_nc(with_mask=with_mask)
    return _NC_CACHE[key]


LAST_RESULTS = None


def kernel(**inputs):
    global LAST_RESULTS
    in_maps, with_mask = host_prepare(
        inputs["x"], inputs["attn_mask"],
        inputs["Wq"], inputs["bq"], inputs["Wk"], inputs["bk"],
        inputs["Wv"], inputs["bv"], inputs["Wo"], inputs["bo"],
    )
    nc = _get_nc(with_mask)
    res = run_bass_kernel_spmd(nc, in_maps, core_ids=list(range(N_CORES)))
    LAST_RESULTS = res
    bo = np.asarray(inputs["bo"], np.float32)
    out = np.zeros((B, S, HIDDEN), np.float32)
    groups = N_CORES // B
    for core in range(N_CORES):
        b = core // groups
        out[b] += res.results[core]["out"]
    out += bo[None, None, :]
    return out
